# revision 20
# baseline (speedup 1.0000x reference)
"""MinCutNet (dense_mincut_pool GNN) forward on 8 Trainium2 NeuronCores.

Self-contained: takes the full inputs of reference.setup_inputs(), shards the
64 graphs across 8 cores (8 graphs each), runs a Bass/Tile kernel per core,
and gathers (log_softmax logits [64,10], mincut_loss, ortho_loss).

Device strategy per core:
  - The transposed adjacency adjT[j,i] (j=dst,i=src, duplicate edges counted)
    is materialized chunk-by-chunk ([128, 1024] bf16) directly in SBUF with
    gpsimd.local_scatter from host-prepared per-row ELL lists.
  - All "wide" tensors are kept channel-major ([ch, node]) so adjacency
    chunks stream through the TensorEngine as matmul rhs:
        P^T = (adj@h0)^T = sum_c h0[c]^T @ adjT[c]      (+ b1 (x) deg outer)
        Q^T = (adj@s)^T  = sum_c s[c]^T  @ adjT[c]
    Biases are applied as per-partition activation biases at PSUM->SBUF copy.
  - Pooling results for each graph are collected as one accumulated matmul
    R^T[80,16] = [h1 | Q | s | deg*s]^T @ s, stacked into RallT[80,128]
    (free-dim stacking), transposed once, and the loss/classifier tail is
    computed batched over all 8 graphs ([128=8*16, *] tiles).
"""

import sys
import types
from contextlib import ExitStack

import numpy as np
import ml_dtypes

import concourse.bass as bass
import concourse.bacc as bacc
import concourse.tile as tile
import concourse.mybir as mybir
from concourse.bass_utils import run_bass_kernel_spmd
from concourse.masks import make_identity

F32 = mybir.dt.float32
BF16 = mybir.dt.bfloat16
I16 = mybir.dt.int16
AX = mybir.AxisListType
OP = mybir.AluOpType
AF = mybir.ActivationFunctionType

B, N, CIN, HID, K, OUT = 64, 1024, 128, 32, 16, 10
NCORES = 8
G = B // NCORES          # graphs per core
NCH = N // 128           # 128-row chunks per graph
EPS = 1e-15

_COMPILED = {}           # L -> compiled Bacc program


# --------------------------------------------------------------------------
# Bass program
# --------------------------------------------------------------------------
def _build(L):
    nc = bacc.Bacc("TRN2", target_bir_lowering=False, debug=False,
                   enable_asserts=False, num_devices=NCORES)

    def inp(name, shape, dt):
        return nc.dram_tensor(name, shape, dt, kind="ExternalInput").ap()

    xsT = inp("xsT", [CIN, G * N], BF16)
    ell_idx = inp("ell_idx", [G, 128, NCH * L], I16)
    ell_dat = inp("ell_dat", [G, 128, NCH * L], BF16)
    deg_row = inp("deg_row", [G, N], BF16)
    dm_ch = inp("dm_ch", [G, 128, 2 * NCH], F32)   # cols 0:8 deg, 8:16 mask
    W1Tb = inp("W1Tb", [CIN, HID], BF16)
    Wcomb1 = inp("Wcomb1", [2 * HID, HID], BF16)  # [Wrel1.T ; Wroot1.T]
    WcombP = inp("WcombP", [2 * HID, K], BF16)    # [(WpWrel1).T ; (WpWroot1).T]
    Wrel2T = inp("Wrel2T", [HID, HID], F32)
    Wroot2T = inp("Wroot2T", [HID, HID], F32)
    W2T = inp("W2T", [HID, HID], F32)
    W3T = inp("W3T", [HID, OUT], F32)
    b1row = inp("b1row", [1, HID], BF16)
    b1col = inp("b1col", [HID, 1], F32)
    brel1col = inp("brel1col", [HID, 1], F32)
    bppbc = inp("bppbc", [128, NCH * K], F32)     # tiled bp + Wp@brel1
    brel2s = inp("brel2s", [HID, 1], F32)     # 16 * brel2
    b2col = inp("b2col", [HID, 1], F32)
    b3col = inp("b3col", [OUT, 1], F32)
    eyem = inp("eyem", [128, K], F32)         # tiled 16x16 eye
    eyem025 = inp("eyem025", [128, K], F32)   # eye / 4
    inv_eye = inp("inv_eye", [128, K], F32)   # 1 - eye
    M8 = inp("M8", [128, G], F32)             # M8[p,g] = (p//16 == g)
    M8T = inp("M8T", [G, 128], F32)

    logits_o = nc.dram_tensor("logits", [G, OUT], F32, kind="ExternalOutput").ap()
    stats_o = nc.dram_tensor("stats", [G, 2], F32, kind="ExternalOutput").ap()

    with tile.TileContext(nc) as tc:
        with ExitStack() as octx:
            cpool = octx.enter_context(tc.tile_pool(name="consts", bufs=1))
            opool = octx.enter_context(tc.tile_pool(name="outer", bufs=1))

            def cload(ap_in, shape):
                t = cpool.tile(shape, F32, tag=f"c{ap_in.tensor.name}")
                nc.sync.dma_start(t[:], ap_in)
                return t

            ident = cpool.tile([128, 128], F32, tag="ident")
            make_identity(nc, ident[:])
            identb = cpool.tile([128, 128], BF16, tag="identb")
            nc.vector.tensor_copy(identb[:], ident[:])

            tW1Tb = cpool.tile([CIN, HID], BF16, tag="cW1Tb")
            nc.sync.dma_start(tW1Tb[:], W1Tb)
            tWcomb1 = cpool.tile([2 * HID, HID], BF16, tag="cWcomb1")
            nc.sync.dma_start(tWcomb1[:], Wcomb1)
            tWcombP = cpool.tile([2 * HID, K], BF16, tag="cWcombP")
            nc.sync.dma_start(tWcombP[:], WcombP)
            tWrel2T = cload(Wrel2T, [HID, HID])
            tWroot2T = cload(Wroot2T, [HID, HID])
            tW2T = cload(W2T, [HID, HID])
            tW3T = cload(W3T, [HID, OUT])
            tb1row = cpool.tile([1, HID], BF16, tag="cb1row")
            nc.sync.dma_start(tb1row[:], b1row)
            tb1col = cload(b1col, [HID, 1])
            tbrel1col = cload(brel1col, [HID, 1])
            tbppbc = cload(bppbc, [128, NCH * K])
            tbrel2s = cload(brel2s, [HID, 1])
            tb2col = cload(b2col, [HID, 1])
            tb3col = cload(b3col, [OUT, 1])
            teyem = cload(eyem, [128, K])
            teyem025 = cload(eyem025, [128, K])
            tinv_eye = cload(inv_eye, [128, K])
            tM8 = cload(M8, [128, G])
            tM8T = cload(M8T, [G, 128])

            RallT = opool.tile([80, 128], F32, tag="RallT")

            # ---------------- main per-graph loop ----------------
            with ExitStack() as ctx:
                sb = ctx.enter_context(tc.tile_pool(name="sb", bufs=3))
                adjp = ctx.enter_context(tc.tile_pool(name="adj", bufs=3 * NCH))
                ellp = ctx.enter_context(tc.tile_pool(name="ell", bufs=1))
                r80p = ctx.enter_context(tc.tile_pool(name="r80", bufs=4))
                ps1 = ctx.enter_context(
                    tc.tile_pool(name="ps1", bufs=4, space="PSUM"))
                psT = ctx.enter_context(
                    tc.tile_pool(name="psT", bufs=2, space="PSUM"))
                psR = ctx.enter_context(
                    tc.tile_pool(name="psR", bufs=2, space="PSUM"))

                # prefetch all per-graph inputs up front (fits in SBUF,
                # keeps POOL/PE from ever waiting on input DMAs)
                prep = ctx.enter_context(tc.tile_pool(name="prep", bufs=1))
                pre = []
                for g in range(G):
                    eit = ellp.tile([128, NCH * L], I16, tag=f"eidx{g}")
                    edt = ellp.tile([128, NCH * L], BF16, tag=f"edat{g}")
                    nc.sync.dma_start(eit[:], ell_idx[g])
                    nc.sync.dma_start(edt[:], ell_dat[g])
                    xbf = prep.tile([128, N], BF16, tag=f"xbf{g}")
                    nc.sync.dma_start(xbf[:], xsT[:, g * N:(g + 1) * N])
                    d_row = prep.tile([1, N], BF16, tag=f"d_row{g}")
                    nc.sync.dma_start(d_row[:], deg_row[g, None, :])
                    dmt = prep.tile([128, 2 * NCH], F32, tag=f"dmt{g}")
                    nc.sync.dma_start(dmt[:], dm_ch[g])
                    pre.append(dict(eit=eit, edt=edt, xbf=xbf, d_row=d_row,
                                    dmt=dmt))

                def front_a(g):
                    """h0 (both layouts) for graph g."""
                    xbf = pre[g]["xbf"]
                    d_row = pre[g]["d_row"]
                    dmt = pre[g]["dmt"]
                    eit = pre[g]["eit"]
                    edt = pre[g]["edt"]

                    # stkoT rows 0:32 = P^T (later), rows 32:64 = h0T
                    stkoT = sb.tile([2 * HID, N], BF16, tag="stkoT")
                    for h in range(2):
                        sl = slice(h * 512, (h + 1) * 512)
                        hp = ps1.tile([HID, 512], F32, tag="ps512",
                                      space="PSUM")
                        nc.tensor.matmul(hp[:], lhsT=tW1Tb[:],
                                         rhs=xbf[:, sl], start=True, stop=True)
                        if h == 0:
                            nc.scalar.activation(stkoT[HID:2 * HID, sl], hp[:],
                                                 AF.Identity, bias=tb1col[:])
                        else:
                            nc.vector.tensor_scalar_add(stkoT[HID:2 * HID, sl],
                                                        hp[:], tb1col[:])
                    h0bf = sb.tile([128, NCH * HID], BF16, tag="h0bf")
                    for c in range(NCH):
                        nb = psT.tile([128, HID], F32, tag="tp", space="PSUM")
                        nc.tensor.matmul(
                            nb[:], lhsT=xbf[:, c * 128:(c + 1) * 128],
                            rhs=tW1Tb[:], start=True, stop=True)
                        nc.vector.tensor_copy(
                            h0bf[:, c * HID:(c + 1) * HID], nb[:])
                    return dict(d_row=d_row, dmt=dmt, eit=eit, edt=edt,
                                stkoT=stkoT, h0bf=h0bf)

                def chunk_phase(g, st, prev):
                    """scatter chunks + P^T for g, interleaved with prev's
                    Q^T so the PE has filler while POOL produces chunks."""
                    adjc = []
                    pt_h0 = ps1.tile([HID, 512], F32, tag="ps512",
                                     space="PSUM")
                    pt_h1 = ps1.tile([HID, 512], F32, tag="ps512",
                                     space="PSUM")
                    pt_hp = [pt_h0, pt_h1]
                    if prev is not None:
                        qt_h0 = ps1.tile([K, 512], F32, tag="ps512",
                                         space="PSUM")
                        qt_h1 = ps1.tile([K, 512], F32, tag="ps512",
                                         space="PSUM")
                        qt_hp = [qt_h0, qt_h1]
                        prev["qt_hp"] = qt_hp
                    for c in range(NCH):
                        at = adjp.tile([128, N], BF16, tag="adj")
                        nc.gpsimd.local_scatter(
                            out_ap=at[:],
                            data_ap=st["edt"][:, c * L:(c + 1) * L],
                            idxs_ap=st["eit"][:, c * L:(c + 1) * L],
                            channels=128, num_elems=N, num_idxs=L)
                        adjc.append(at)
                        if prev is not None:
                            padj = prev["adjc"][c]
                            for h in range(2):
                                sl = slice(h * 512, (h + 1) * 512)
                                nc.tensor.matmul(
                                    qt_hp[h][:],
                                    lhsT=prev["sbf"][:, c * K:(c + 1) * K],
                                    rhs=padj[:, sl], start=(c == 0),
                                    stop=(c == NCH - 1),
                                    skip_group_check=True)
                        for h in range(2):
                            sl = slice(h * 512, (h + 1) * 512)
                            nc.tensor.matmul(
                                pt_hp[h][:],
                                lhsT=st["h0bf"][:, c * HID:(c + 1) * HID],
                                rhs=at[:, sl], start=(c == 0), stop=False,
                                skip_group_check=True)
                    for h in range(2):
                        sl = slice(h * 512, (h + 1) * 512)
                        nc.tensor.matmul(pt_hp[h][:], lhsT=tb1row[:],
                                         rhs=st["d_row"][:, sl], start=False,
                                         stop=True, skip_group_check=True)
                        if h == 0:
                            nc.scalar.copy(st["stkoT"][0:HID, sl], pt_hp[h][:])
                        else:
                            nc.vector.tensor_copy(st["stkoT"][0:HID, sl],
                                                  pt_hp[h][:])
                    st["adjc"] = adjc

                def qt_out(prev):
                    """copy prev's Q^T psum into stk rows 32:48."""
                    qt_hp = prev["qt_hp"]
                    stk = prev["stk"]
                    nc.scalar.copy(stk[HID:HID + K, 0:512], qt_hp[0][:])
                    nc.vector.tensor_copy(stk[HID:HID + K, 512:1024],
                                          qt_hp[1][:])

                def rt_phase(g, prev):
                    """node-major transposes + pooled R^T for prev graph."""
                    stk, sbf, dmt = prev["stk"], prev["sbf"], prev["dmt"]
                    RT_ps = psR.tile([80, K], F32, tag="psr", space="PSUM")
                    tpb = psT.tile([128, NCH * 48], BF16, tag="tp",
                                   space="PSUM")
                    for c in range(NCH):
                        nc.tensor.transpose(
                            tpb[:, c * 48:(c + 1) * 48],
                            stk[:, c * 128:(c + 1) * 128],
                            identb[0:48, 0:48])
                    for c in range(NCH):
                        r80 = r80p.tile([128, 80], BF16, tag="rhs80")
                        tp = tpb[:, c * 48:(c + 1) * 48]
                        nc.vector.tensor_scalar_mul(
                            r80[:, 0:HID], tp[:, 0:HID],
                            dmt[:, NCH + c:NCH + c + 1])
                        nc.vector.tensor_copy(
                            r80[:, HID:HID + K], tp[:, HID:HID + K])
                        nc.vector.tensor_copy(r80[:, HID + K:HID + 2 * K],
                                              sbf[:, c * K:(c + 1) * K])
                        nc.scalar.mul(r80[:, HID + 2 * K:HID + 3 * K],
                                      sbf[:, c * K:(c + 1) * K],
                                      dmt[:, c:c + 1])
                        nc.tensor.matmul(
                            RT_ps[:], lhsT=r80[:],
                            rhs=sbf[:, c * K:(c + 1) * K],
                            start=(c == 0), stop=(c == NCH - 1))
                    nc.vector.tensor_copy(
                        RallT[:, g * K:(g + 1) * K], RT_ps[:])

                def front_b(g, st):
                    """h1 (into stk), node-major s_pre, softmax, for g."""
                    stkoT, dmt = st["stkoT"], st["dmt"]
                    stk = sb.tile([48, N], BF16, tag="stk")
                    for h in range(2):
                        sl = slice(h * 512, (h + 1) * 512)
                        hp = ps1.tile([HID, 512], F32, tag="ps512",
                                      space="PSUM")
                        nc.tensor.matmul(hp[:], lhsT=tWcomb1[:],
                                         rhs=stkoT[:, sl], start=True,
                                         stop=True)
                        if h == 0:
                            nc.scalar.activation(stk[0:HID, sl], hp[:],
                                                 AF.Identity,
                                                 bias=tbrel1col[:])
                        else:
                            nc.vector.tensor_scalar_add(stk[0:HID, sl], hp[:],
                                                        tbrel1col[:])
                    spre = sb.tile([128, NCH * K], F32, tag="spre")
                    tpa = psT.tile([128, NCH * K], F32, tag="tp", space="PSUM")
                    for c in range(NCH):
                        nc.tensor.matmul(
                            tpa[:, c * K:(c + 1) * K],
                            lhsT=stkoT[:, c * 128:(c + 1) * 128],
                            rhs=tWcombP[:], start=True, stop=True)
                    nc.vector.tensor_tensor(spre[:], tpa[:], tbppbc[:],
                                            op=OP.add)
                    gview = spre[:].rearrange("p (c k) -> p c k", k=K)
                    mx = sb.tile([128, NCH], F32, tag="mx")
                    nc.vector.tensor_reduce(mx[:], gview, axis=AX.X, op=OP.max)
                    s_n = sb.tile([128, NCH * K], F32, tag="s_n")
                    nc.vector.tensor_tensor(
                        out=s_n[:].rearrange("p (c k) -> p c k", k=K),
                        in0=gview,
                        in1=mx[:].rearrange("p (c o) -> p c o", o=1)
                        .to_broadcast([128, NCH, K]),
                        op=OP.subtract)
                    nc.scalar.activation(s_n[:], s_n[:], AF.Exp)
                    sm = sb.tile([128, NCH], F32, tag="sm")
                    nc.vector.tensor_reduce(
                        sm[:], s_n[:].rearrange("p (c k) -> p c k", k=K),
                        axis=AX.X, op=OP.add)
                    rcp = sb.tile([128, NCH], F32, tag="rcp")
                    nc.vector.reciprocal(rcp[:], sm[:])
                    nc.vector.tensor_tensor(rcp[:], rcp[:],
                                            dmt[:, NCH:2 * NCH], op=OP.mult)
                    sbf = sb.tile([128, NCH * K], BF16, tag="sbf")
                    nc.vector.tensor_tensor(
                        out=sbf[:].rearrange("p (c k) -> p c k", k=K),
                        in0=s_n[:].rearrange("p (c k) -> p c k", k=K),
                        in1=rcp[:].rearrange("p (c o) -> p c o", o=1)
                        .to_broadcast([128, NCH, K]),
                        op=OP.mult)
                    st["stk"] = stk
                    st["sbf"] = sbf

                # --- chunk-interleaved software pipeline over graphs ---
                prev = None
                for g in range(G):
                    st = front_a(g)
                    chunk_phase(g, st, prev)
                    if prev is not None:
                        qt_out(prev)
                        rt_phase(g - 1, prev)
                    front_b(g, st)
                    prev = st
                # drain: last graph's QT + RT
                qt_h0 = ps1.tile([K, 512], F32, tag="ps512", space="PSUM")
                qt_h1 = ps1.tile([K, 512], F32, tag="ps512", space="PSUM")
                qt_hp = [qt_h0, qt_h1]
                prev["qt_hp"] = qt_hp
                for c in range(NCH):
                    for h in range(2):
                        sl = slice(h * 512, (h + 1) * 512)
                        nc.tensor.matmul(
                            qt_hp[h][:],
                            lhsT=prev["sbf"][:, c * K:(c + 1) * K],
                            rhs=prev["adjc"][c][:, sl], start=(c == 0),
                            stop=(c == NCH - 1), skip_group_check=True)
                qt_out(prev)
                rt_phase(G - 1, prev)

            # ---------------- batched tail over the G graphs ----------------
            with ExitStack() as ctx:
                sb = ctx.enter_context(tc.tile_pool(name="tail", bufs=1))
                ps = ctx.enter_context(
                    tc.tile_pool(name="tailps", bufs=4, space="PSUM"))

                rl_ps = ps.tile([128, 80], F32, tag="tps", space="PSUM")
                nc.tensor.transpose(rl_ps[:], RallT[:], ident[0:80, 0:80])
                Rall = sb.tile([128, 80], F32, tag="Rall")
                nc.vector.tensor_copy(Rall[:], rl_ps[:])
                r_out = Rall[:, 0:HID]            # out^T [16g+k, ch]
                r_oadj = Rall[:, HID:HID + K]     # out_adj [16g+k, l]
                r_ss = Rall[:, HID + K:HID + 2 * K]
                r_sds = Rall[:, HID + 2 * K:HID + 3 * K]

                # --- losses ---
                st3 = sb.tile([128, 3], F32, tag="st3")
                t16 = sb.tile([128, K], F32, tag="t16")
                nc.vector.tensor_tensor(t16[:], r_oadj, teyem[:], op=OP.mult)
                nc.vector.tensor_reduce(st3[:, 0:1], t16[:], axis=AX.X, op=OP.add)
                nc.vector.tensor_tensor(t16[:], r_sds, teyem[:], op=OP.mult)
                nc.vector.tensor_reduce(st3[:, 1:2], t16[:], axis=AX.X, op=OP.add)
                nc.scalar.activation(t16[:], r_ss, AF.Square,
                                     accum_out=st3[:, 2:3])
                red_ps = ps.tile([G, 3], F32, tag="tps", space="PSUM")
                nc.tensor.matmul(red_ps[:], lhsT=tM8[:], rhs=st3[:],
                                 start=True, stop=True)
                red = sb.tile([G, 3], F32, tag="red_sb")
                nc.vector.tensor_copy(red[:], red_ps[:])
                rden = sb.tile([G, 1], F32, tag="rden")
                nc.vector.reciprocal(rden[:], red[:, 1:2])
                lossg = sb.tile([G, 2], F32, tag="lossg")
                nc.vector.tensor_tensor(lossg[:, 0:1], red[:, 0:1], rden[:],
                                        op=OP.mult)
                nc.vector.tensor_scalar_mul(lossg[:, 0:1], lossg[:, 0:1], -1.0)
                ssn = sb.tile([G, 1], F32, tag="ssn")
                nc.scalar.sqrt(ssn[:], red[:, 2:3])
                invssn = sb.tile([G, 1], F32, tag="invssn")
                nc.vector.reciprocal(invssn[:], ssn[:])
                bc_ps = ps.tile([128, 1], F32, tag="tps", space="PSUM")
                nc.tensor.matmul(bc_ps[:], lhsT=tM8T[:], rhs=invssn[:],
                                 start=True, stop=True)
                invbc = sb.tile([128, 1], F32, tag="invbc")
                nc.vector.tensor_copy(invbc[:], bc_ps[:])
                E = sb.tile([128, K], F32, tag="E")
                nc.vector.tensor_scalar_mul(E[:], r_ss, invbc[:])
                nc.vector.tensor_tensor(E[:], E[:], teyem025[:], op=OP.subtract)
                v3 = sb.tile([128, 1], F32, tag="v3")
                nc.scalar.activation(E[:], E[:], AF.Square, accum_out=v3[:])
                o2_ps = ps.tile([G, 1], F32, tag="tps", space="PSUM")
                nc.tensor.matmul(o2_ps[:], lhsT=tM8[:], rhs=v3[:],
                                 start=True, stop=True)
                nc.scalar.sqrt(lossg[:, 1:2], o2_ps[:])
                nc.sync.dma_start(stats_o[:, :], lossg[:])

                # --- h2 chain / classifier ---
                oadj = sb.tile([128, K], F32, tag="oadj")
                nc.vector.tensor_tensor(oadj[:], r_oadj, tinv_eye[:], op=OP.mult)
                rs = sb.tile([128, 1], F32, tag="rs")
                nc.vector.tensor_reduce(rs[:], oadj[:], axis=AX.X, op=OP.add)
                dsq = sb.tile([128, 1], F32, tag="dsq")
                nc.scalar.sqrt(dsq[:], rs[:])
                nc.vector.tensor_scalar_add(dsq[:], dsq[:], EPS)
                invd = sb.tile([128, 1], F32, tag="invd")
                nc.vector.reciprocal(invd[:], dsq[:])
                oadj1 = sb.tile([128, K], F32, tag="oadj1")
                nc.vector.tensor_scalar_mul(oadj1[:], oadj[:], invd[:])
                u_ps = ps.tile([G, K], F32, tag="tps", space="PSUM")
                nc.tensor.matmul(u_ps[:], lhsT=tM8[:], rhs=oadj1[:],
                                 start=True, stop=True)
                u_sb = sb.tile([G, K], F32, tag="u_sb")
                nc.vector.tensor_copy(u_sb[:], u_ps[:])
                bc2_ps = ps.tile([128, K], F32, tag="tps", space="PSUM")
                nc.tensor.matmul(bc2_ps[:], lhsT=tM8T[:], rhs=u_sb[:],
                                 start=True, stop=True)
                ubct = sb.tile([128, K], F32, tag="ubct")
                nc.vector.tensor_tensor(ubct[:], bc2_ps[:], teyem[:], op=OP.mult)
                ubc = sb.tile([128, 1], F32, tag="ubc")
                nc.vector.tensor_reduce(ubc[:], ubct[:], axis=AX.X, op=OP.add)
                U = sb.tile([128, G], F32, tag="U")
                nc.vector.tensor_scalar_mul(U[:], tM8[:], ubc[:])
                outsc = sb.tile([128, HID], F32, tag="outsc")
                nc.vector.tensor_scalar_mul(outsc[:], r_out, invd[:])
                woutT_ps = ps.tile([HID, G], F32, tag="tps", space="PSUM")
                nc.tensor.matmul(woutT_ps[:], lhsT=outsc[:], rhs=U[:],
                                 start=True, stop=True)
                woutT = sb.tile([HID, G], F32, tag="woutT")
                nc.vector.tensor_copy(woutT[:], woutT_ps[:])
                colsT_ps = ps.tile([HID, G], F32, tag="tps", space="PSUM")
                nc.tensor.matmul(colsT_ps[:], lhsT=r_out, rhs=tM8[:],
                                 start=True, stop=True)
                colsT = sb.tile([HID, G], F32, tag="colsT")
                nc.vector.tensor_copy(colsT[:], colsT_ps[:])
                gT_ps = ps.tile([HID, G], F32, tag="tps", space="PSUM")
                nc.tensor.matmul(gT_ps[:], lhsT=tWrel2T[:], rhs=woutT[:],
                                 start=True, stop=False)
                nc.tensor.matmul(gT_ps[:], lhsT=tWroot2T[:], rhs=colsT[:],
                                 start=False, stop=True)
                gTs = sb.tile([HID, G], F32, tag="gTs")
                nc.scalar.activation(gTs[:], gT_ps[:], AF.Identity,
                                     bias=tbrel2s[:])
                preT_ps = ps.tile([HID, G], F32, tag="tps", space="PSUM")
                nc.tensor.matmul(preT_ps[:], lhsT=tW2T[:], rhs=gTs[:],
                                 start=True, stop=True)
                preTs = sb.tile([HID, G], F32, tag="preTs")
                nc.scalar.activation(preTs[:], preT_ps[:], AF.Relu,
                                     bias=tb2col[:])
                logT_ps = ps.tile([OUT, G], F32, tag="tps", space="PSUM")
                nc.tensor.matmul(logT_ps[:], lhsT=tW3T[:], rhs=preTs[:],
                                 start=True, stop=True)
                logTs = sb.tile([OUT, G], F32, tag="logTs")
                nc.scalar.activation(logTs[:], logT_ps[:], AF.Identity,
                                     bias=tb3col[:])
                lgT_ps = ps.tile([G, OUT], F32, tag="tps", space="PSUM")
                nc.tensor.transpose(lgT_ps[:], logTs[:], ident[0:OUT, 0:OUT])
                lgs = sb.tile([G, OUT], F32, tag="lgs")
                nc.vector.tensor_copy(lgs[:], lgT_ps[:])
                lmx = sb.tile([G, 1], F32, tag="lmx")
                nc.vector.tensor_reduce(lmx[:], lgs[:], axis=AX.X, op=OP.max)
                xm = sb.tile([G, OUT], F32, tag="xm")
                nc.vector.tensor_scalar(xm[:], lgs[:], lmx[:], None,
                                        op0=OP.subtract)
                ex = sb.tile([G, OUT], F32, tag="ex")
                esum = sb.tile([G, 1], F32, tag="esum")
                nc.scalar.activation(ex[:], xm[:], AF.Exp, accum_out=esum[:])
                lnz = sb.tile([G, 1], F32, tag="lnz")
                nc.scalar.activation(lnz[:], esum[:], AF.Ln)
                lout = sb.tile([G, OUT], F32, tag="lout")
                nc.vector.tensor_scalar(lout[:], xm[:], lnz[:], None,
                                        op0=OP.subtract)
                nc.sync.dma_start(logits_o[:, :], lout[:])

    nc.compile()
    return nc


# --------------------------------------------------------------------------
# Host-side prep
# --------------------------------------------------------------------------
def _prep(x, edge_index, batch):
    x = np.asarray(x, np.float32)
    batch = np.asarray(batch, np.int64)
    edge_index = np.asarray(edge_index, np.int64)
    total = x.shape[0]

    counts = np.bincount(batch, minlength=B)
    starts = np.concatenate([[0], np.cumsum(counts)[:-1]]).astype(np.int64)
    pos = np.arange(total, dtype=np.int64) - starts[batch]

    if counts.min() == N and counts.max() == N:
        xd = x
        mask = np.ones((B, N), np.float32)
    else:
        ok = pos < N
        tgt = batch * N + pos
        xd = np.zeros((B * N, CIN), np.float32)
        xd[tgt[ok]] = x[ok]
        mask = np.zeros(B * N, np.float32)
        mask[tgt[ok]] = 1.0
        mask = mask.reshape(B, N)

    src, dst = edge_index[0], edge_index[1]
    b_e = batch[src]
    psrc, pdst = pos[src], pos[dst]
    ev = (psrc < N) & (pdst < N)
    if not ev.all():
        b_e, psrc, pdst = b_e[ev], psrc[ev], pdst[ev]

    deg = np.bincount(b_e * N + psrc, minlength=B * N)
    deg = deg.astype(np.float32).reshape(B, N)

    key = ((b_e * N + pdst) * N + psrc).astype(np.int64)
    key.sort()
    first = np.empty(len(key), bool)
    first[0] = True
    np.not_equal(key[1:], key[:-1], out=first[1:])
    uq_pos = np.flatnonzero(first)
    uq = key[uq_pos]
    cnt = np.diff(np.append(uq_pos, len(key))).astype(np.float32)

    row = (uq >> 10).astype(np.int64)     # b*N + pdst
    col = (uq & (N - 1)).astype(np.int16)
    rc = np.bincount(row, minlength=B * N)
    L = int(max(rc.max(), 2))
    L += L % 2
    rstart = np.concatenate([[0], np.cumsum(rc)[:-1]])
    offs = np.arange(len(uq)) - rstart[row]
    ell_idx = np.full((B * N, L), -1, np.int16)
    ell_dat = np.zeros((B * N, L), ml_dtypes.bfloat16)
    ell_idx[row, offs] = col
    ell_dat[row, offs] = cnt.astype(ml_dtypes.bfloat16)

    return xd, mask, deg, ell_idx, ell_dat, L


def _weight_consts(ins):
    f32 = np.float32
    eye = np.tile(np.eye(K, dtype=f32), (128 // K, 1))
    m8 = np.zeros((128, G), f32)
    m8[np.arange(128), np.arange(128) // K] = 1.0
    c = dict(

        Wrel2T=ins["Wrel2"].T,
        Wroot2T=ins["Wroot2"].T,
        W2T=ins["W2"].T,
        W3T=ins["W3"].T,

        b1col=ins["b1"][:, None],
        brel1col=ins["brel1"][:, None],
        bppbc=np.tile(ins["bp"] + ins["Wp"] @ ins["brel1"], (128, NCH)),
        brel2s=(K * ins["brel2"])[:, None],
        b2col=ins["b2"][:, None],
        b3col=ins["b3"][:, None],
        eyem=eye,
        eyem025=eye * 0.25,
        inv_eye=1.0 - eye,
        M8=m8,
        M8T=m8.T,
    )
    out = {k: np.ascontiguousarray(np.asarray(v, f32)) for k, v in c.items()}
    bf = ml_dtypes.bfloat16
    out["b1row"] = np.ascontiguousarray(ins["b1"][None, :].astype(bf))
    out["Wcomb1"] = np.ascontiguousarray(np.concatenate(
        [ins["Wrel1"].T, ins["Wroot1"].T], axis=0).astype(bf))
    out["WcombP"] = np.ascontiguousarray(np.concatenate(
        [(ins["Wp"] @ ins["Wrel1"]).T,
         (ins["Wp"] @ ins["Wroot1"]).T], axis=0).astype(bf))
    return out


def _install_profhook():
    try:
        import antenv
        if not hasattr(antenv, "axon_hooks"):
            mod = types.ModuleType("antenv.axon_hooks")
            mod._hook = None
            mod.set_axon_ntff_profile_hook = lambda h: setattr(mod, "_hook", h)
            mod.get_axon_ntff_profile_hook = lambda: mod._hook
            sys.modules["antenv.axon_hooks"] = mod
            antenv.axon_hooks = mod
        from trn_agent_boot.trn_boot import _ntff_profile_via_ctypes
        hook = _ntff_profile_via_ctypes("/opt/axon/libaxon_pjrt.so")
        if hook is not None:
            sys.modules["antenv.axon_hooks"].set_axon_ntff_profile_hook(hook)
            return True
    except Exception:
        pass
    return False


def _run(inputs, trace=False):
    ins = {k: np.asarray(v) for k, v in inputs.items()}
    xd, mask, deg, ell_idx, ell_dat, L = _prep(
        ins["x"], ins["edge_index"], ins["batch"])
    consts = _weight_consts(ins)

    if L not in _COMPILED:
        _COMPILED[L] = _build(L)
    nc = _COMPILED[L]

    xdT = np.ascontiguousarray(xd.T)                      # [CIN, B*N]
    # ELL rows for graph g, partition p, chunk c at [g, p, c*L:(c+1)*L]
    ell_idx = np.ascontiguousarray(
        ell_idx.reshape(B, NCH, 128, L).transpose(0, 2, 1, 3).reshape(
            B, 128, NCH * L))
    ell_dat = np.ascontiguousarray(
        ell_dat.reshape(B, NCH, 128, L).transpose(0, 2, 1, 3).reshape(
            B, 128, NCH * L))
    dm = np.concatenate(
        [deg.reshape(B, NCH, 128).transpose(0, 2, 1),
         mask.reshape(B, NCH, 128).transpose(0, 2, 1)], axis=2)
    dm = np.ascontiguousarray(dm)                         # [B, 128, 16]

    in_maps = []
    for core in range(NCORES):
        gsl = slice(core * G, (core + 1) * G)
        m = dict(
            xsT=np.ascontiguousarray(
                xdT[:, core * G * N:(core + 1) * G * N].astype(
                    ml_dtypes.bfloat16)),
            W1Tb=np.ascontiguousarray(
                ins["W1"].T.astype(ml_dtypes.bfloat16)),
            ell_idx=ell_idx[gsl],
            ell_dat=ell_dat[gsl],
            deg_row=np.ascontiguousarray(deg[gsl].astype(ml_dtypes.bfloat16)),
            dm_ch=dm[gsl],
            **consts,
        )
        in_maps.append(m)

    if trace:
        _install_profhook()
    res = run_bass_kernel_spmd(nc, in_maps, core_ids=list(range(NCORES)),
                               trace=trace)
    logits = np.concatenate([r["logits"] for r in res.results], axis=0)
    stats = np.concatenate([r["stats"] for r in res.results], axis=0)
    mincut = np.float32(stats[:, 0].mean())
    ortho = np.float32(stats[:, 1].mean())
    return (logits, mincut, ortho), res


def kernel(**inputs):
    out, _ = _run(inputs, trace=False)
    return out


# revision 23
# speedup vs baseline: 1.2298x; 1.2298x over previous
"""MinCutNet (dense_mincut_pool GNN) forward on 8 Trainium2 NeuronCores.

Self-contained: takes the full inputs of reference.setup_inputs(), shards the
64 graphs across 8 cores (8 graphs each), runs a Bass/Tile kernel per core,
and gathers (log_softmax logits [64,10], mincut_loss, ortho_loss).

Device strategy per core:
  - The transposed adjacency adjT[j,i] (j=dst,i=src, duplicate edges counted)
    is materialized chunk-by-chunk ([128, 1024] bf16) directly in SBUF with
    gpsimd.local_scatter from host-prepared per-row ELL lists.
  - All "wide" tensors are kept channel-major ([ch, node]) so adjacency
    chunks stream through the TensorEngine as matmul rhs:
        P^T = (adj@h0)^T = sum_c h0[c]^T @ adjT[c]      (+ b1 (x) deg outer)
        Q^T = (adj@s)^T  = sum_c s[c]^T  @ adjT[c]
    Biases are applied as per-partition activation biases at PSUM->SBUF copy.
  - Pooling results for each graph are collected as one accumulated matmul
    R^T[80,16] = [h1 | Q | s | deg*s]^T @ s, stacked into RallT[80,128]
    (free-dim stacking), transposed once, and the loss/classifier tail is
    computed batched over all 8 graphs ([128=8*16, *] tiles).
"""

import sys
import types
from contextlib import ExitStack

import numpy as np
import ml_dtypes

import concourse.bass as bass
import concourse.bacc as bacc
import concourse.tile as tile
import concourse.mybir as mybir
from concourse.bass_utils import run_bass_kernel_spmd
from concourse.masks import make_identity

F32 = mybir.dt.float32
BF16 = mybir.dt.bfloat16
I16 = mybir.dt.int16
AX = mybir.AxisListType
OP = mybir.AluOpType
AF = mybir.ActivationFunctionType

B, N, CIN, HID, K, OUT = 64, 1024, 128, 32, 16, 10
NCORES = 8
G = B // NCORES          # graphs per core
NCH = N // 128           # 128-row chunks per graph
EPS = 1e-15

_COMPILED = {}           # L -> compiled Bacc program


# --------------------------------------------------------------------------
# Bass program
# --------------------------------------------------------------------------
def _build(L):
    nc = bacc.Bacc("TRN2", target_bir_lowering=False, debug=False,
                   enable_asserts=False, num_devices=NCORES)

    def inp(name, shape, dt):
        return nc.dram_tensor(name, shape, dt, kind="ExternalInput").ap()

    xsT = inp("xsT", [CIN, G * N], BF16)
    ell_idx = inp("ell_idx", [G, 128, NCH * L], I16)
    ell_dat = inp("ell_dat", [G, 128, NCH * L], BF16)
    deg_row = inp("deg_row", [G, N], BF16)
    dm_ch = inp("dm_ch", [G, 128, 2 * NCH], F32)   # cols 0:8 deg, 8:16 mask
    W1Tb = inp("W1Tb", [CIN, HID], BF16)
    Wcomb1 = inp("Wcomb1", [2 * HID, HID], BF16)  # [Wrel1.T ; Wroot1.T]
    WcombP = inp("WcombP", [2 * HID, K], BF16)    # [(WpWrel1).T ; (WpWroot1).T]
    Wrel2T = inp("Wrel2T", [HID, HID], F32)
    Wroot2T = inp("Wroot2T", [HID, HID], F32)
    W2T = inp("W2T", [HID, HID], F32)
    W3T = inp("W3T", [HID, OUT], F32)
    b1row = inp("b1row", [1, HID], BF16)
    b1col = inp("b1col", [HID, 1], F32)
    brel1col = inp("brel1col", [HID, 1], F32)
    bppbc = inp("bppbc", [128, NCH * K], F32)     # tiled bp + Wp@brel1
    brel2s = inp("brel2s", [HID, 1], F32)     # 16 * brel2
    b2col = inp("b2col", [HID, 1], F32)
    b3col = inp("b3col", [OUT, 1], F32)
    eyem = inp("eyem", [128, K], F32)         # tiled 16x16 eye
    eyem025 = inp("eyem025", [128, K], F32)   # eye / 4
    inv_eye = inp("inv_eye", [128, K], F32)   # 1 - eye
    M8 = inp("M8", [128, G], F32)             # M8[p,g] = (p//16 == g)
    M8T = inp("M8T", [G, 128], F32)

    logits_o = nc.dram_tensor("logits", [G, OUT], F32, kind="ExternalOutput").ap()
    stats_o = nc.dram_tensor("stats", [G, 2], F32, kind="ExternalOutput").ap()

    with tile.TileContext(nc) as tc:
        with ExitStack() as octx:
            cpool = octx.enter_context(tc.tile_pool(name="consts", bufs=1))
            opool = octx.enter_context(tc.tile_pool(name="outer", bufs=1))

            def cload(ap_in, shape):
                t = cpool.tile(shape, F32, tag=f"c{ap_in.tensor.name}")
                nc.sync.dma_start(t[:], ap_in)
                return t

            ident = cpool.tile([128, 128], F32, tag="ident")
            make_identity(nc, ident[:])
            identb = cpool.tile([128, 128], BF16, tag="identb")
            nc.vector.tensor_copy(identb[:], ident[:])

            tW1Tb = cpool.tile([CIN, HID], BF16, tag="cW1Tb")
            nc.sync.dma_start(tW1Tb[:], W1Tb)
            tWcomb1 = cpool.tile([2 * HID, HID], BF16, tag="cWcomb1")
            nc.sync.dma_start(tWcomb1[:], Wcomb1)
            tWcombP = cpool.tile([2 * HID, K], BF16, tag="cWcombP")
            nc.sync.dma_start(tWcombP[:], WcombP)
            tWrel2T = cload(Wrel2T, [HID, HID])
            tWroot2T = cload(Wroot2T, [HID, HID])
            tW2T = cload(W2T, [HID, HID])
            tW3T = cload(W3T, [HID, OUT])
            tb1row = cpool.tile([1, HID], BF16, tag="cb1row")
            nc.sync.dma_start(tb1row[:], b1row)
            tb1col = cload(b1col, [HID, 1])
            tbrel1col = cload(brel1col, [HID, 1])
            tbppbc = cload(bppbc, [128, NCH * K])
            tbrel2s = cload(brel2s, [HID, 1])
            tb2col = cload(b2col, [HID, 1])
            tb3col = cload(b3col, [OUT, 1])
            teyem = cload(eyem, [128, K])
            teyem025 = cload(eyem025, [128, K])
            tinv_eye = cload(inv_eye, [128, K])
            tM8 = cload(M8, [128, G])
            tM8T = cload(M8T, [G, 128])

            RallT = opool.tile([112, 128], F32, tag="RallT")

            # ---------------- main per-graph loop ----------------
            with ExitStack() as ctx:
                sb = ctx.enter_context(tc.tile_pool(name="sb", bufs=3))
                adjp = ctx.enter_context(tc.tile_pool(name="adj", bufs=3 * NCH))
                ellp = ctx.enter_context(tc.tile_pool(name="ell", bufs=1))
                r80p = ctx.enter_context(tc.tile_pool(name="r80", bufs=4))
                ps1 = ctx.enter_context(
                    tc.tile_pool(name="ps1", bufs=4, space="PSUM"))
                psT = ctx.enter_context(
                    tc.tile_pool(name="psT", bufs=2, space="PSUM"))
                psR = ctx.enter_context(
                    tc.tile_pool(name="psR", bufs=1, space="PSUM"))

                # prefetch all per-graph inputs up front (fits in SBUF,
                # keeps POOL/PE from ever waiting on input DMAs)
                prep = ctx.enter_context(tc.tile_pool(name="prep", bufs=1))
                pre = []
                for g in range(G):
                    eit = ellp.tile([128, NCH * L], I16, tag=f"eidx{g}")
                    edt = ellp.tile([128, NCH * L], BF16, tag=f"edat{g}")
                    nc.sync.dma_start(eit[:], ell_idx[g])
                    nc.sync.dma_start(edt[:], ell_dat[g])
                    xbf = prep.tile([128, N], BF16, tag=f"xbf{g}")
                    nc.sync.dma_start(xbf[:], xsT[:, g * N:(g + 1) * N])
                    d_row = prep.tile([1, N], BF16, tag=f"d_row{g}")
                    nc.sync.dma_start(d_row[:], deg_row[g, None, :])
                    dmt = prep.tile([128, 2 * NCH], F32, tag=f"dmt{g}")
                    nc.sync.dma_start(dmt[:], dm_ch[g])
                    pre.append(dict(eit=eit, edt=edt, xbf=xbf, d_row=d_row,
                                    dmt=dmt))

                def front_a(g):
                    """h0 (both layouts) for graph g."""
                    xbf = pre[g]["xbf"]
                    d_row = pre[g]["d_row"]
                    dmt = pre[g]["dmt"]
                    eit = pre[g]["eit"]
                    edt = pre[g]["edt"]

                    # stkoT rows 0:32 = P^T (later), rows 32:64 = h0T
                    stkoT = sb.tile([2 * HID, N], BF16, tag="stkoT")
                    for h in range(2):
                        sl = slice(h * 512, (h + 1) * 512)
                        hp = ps1.tile([HID, 512], F32, tag="ps512",
                                      space="PSUM")
                        nc.tensor.matmul(hp[:], lhsT=tW1Tb[:],
                                         rhs=xbf[:, sl], start=True, stop=True)
                        if h == 0:
                            nc.scalar.activation(stkoT[HID:2 * HID, sl], hp[:],
                                                 AF.Identity, bias=tb1col[:])
                        else:
                            nc.vector.tensor_scalar_add(stkoT[HID:2 * HID, sl],
                                                        hp[:], tb1col[:])
                    h0bf = sb.tile([128, NCH * HID], BF16, tag="h0bf")
                    nba = psT.tile([128, NCH * HID], F32, tag="tp",
                                   space="PSUM")
                    for c in range(NCH):
                        nc.tensor.matmul(
                            nba[:, c * HID:(c + 1) * HID],
                            lhsT=xbf[:, c * 128:(c + 1) * 128],
                            rhs=tW1Tb[:], start=True, stop=True)
                    nc.vector.tensor_copy(h0bf[:], nba[:])
                    return dict(d_row=d_row, dmt=dmt, eit=eit, edt=edt,
                                stkoT=stkoT, h0bf=h0bf)

                def chunk_phase(g, st, prev):
                    """scatter chunks + P^T for g, interleaved with prev's
                    Q^T so the PE has filler while POOL produces chunks."""
                    adjc = []
                    pt_h0 = ps1.tile([HID, 512], F32, tag="ps512",
                                     space="PSUM")
                    pt_h1 = ps1.tile([HID, 512], F32, tag="ps512",
                                     space="PSUM")
                    pt_hp = [pt_h0, pt_h1]
                    if prev is not None:
                        qt_h0 = ps1.tile([K, 512], F32, tag="ps512",
                                         space="PSUM")
                        qt_h1 = ps1.tile([K, 512], F32, tag="ps512",
                                         space="PSUM")
                        qt_hp = [qt_h0, qt_h1]
                        prev["qt_hp"] = qt_hp
                    for c in range(NCH):
                        at = adjp.tile([128, N], BF16, tag="adj")
                        nc.gpsimd.local_scatter(
                            out_ap=at[:],
                            data_ap=st["edt"][:, c * L:(c + 1) * L],
                            idxs_ap=st["eit"][:, c * L:(c + 1) * L],
                            channels=128, num_elems=N, num_idxs=L)
                        adjc.append(at)
                        if prev is not None:
                            padj = prev["adjc"][c]
                            for h in range(2):
                                sl = slice(h * 512, (h + 1) * 512)
                                nc.tensor.matmul(
                                    qt_hp[h][:],
                                    lhsT=prev["sbf"][:, c * K:(c + 1) * K],
                                    rhs=padj[:, sl], start=(c == 0),
                                    stop=(c == NCH - 1),
                                    skip_group_check=True)
                        for h in range(2):
                            sl = slice(h * 512, (h + 1) * 512)
                            nc.tensor.matmul(
                                pt_hp[h][:],
                                lhsT=st["h0bf"][:, c * HID:(c + 1) * HID],
                                rhs=at[:, sl], start=(c == 0), stop=False,
                                skip_group_check=True)
                    for h in range(2):
                        sl = slice(h * 512, (h + 1) * 512)
                        nc.tensor.matmul(pt_hp[h][:], lhsT=tb1row[:],
                                         rhs=st["d_row"][:, sl], start=False,
                                         stop=True, skip_group_check=True)
                        if h == 0:
                            nc.scalar.copy(st["stkoT"][0:HID, sl], pt_hp[h][:])
                        else:
                            nc.vector.tensor_copy(st["stkoT"][0:HID, sl],
                                                  pt_hp[h][:])
                    st["adjc"] = adjc

                def qt_out(prev):
                    """copy prev's Q^T psum into stk rows 32:48."""
                    qt_hp = prev["qt_hp"]
                    stk = prev["stk"]
                    nc.scalar.copy(stk[HID:HID + K, 0:512], qt_hp[0][:])
                    nc.vector.tensor_copy(stk[HID:HID + K, 512:1024],
                                          qt_hp[1][:])

                def rt_phase(g, prev):
                    """node-major transposes + pooled R^T for prev graph.
                    R rows: 0:32 h1 | 32:48 Q | 64:80 s | 96:112 deg*s
                    (no mask needed on h1: s is already masked, mask^2=mask)"""
                    stk, sbf, dmt = prev["stk"], prev["sbf"], prev["dmt"]
                    RT_ps = psR.tile([80, K], F32, tag="psr", space="PSUM")
                    RT_ps2 = psR.tile([K, K], F32, tag="psr2", space="PSUM")
                    tpb = psT.tile([128, NCH * 48], BF16, tag="tp",
                                   space="PSUM")
                    for c in range(NCH):
                        nc.tensor.transpose(
                            tpb[:, c * 48:(c + 1) * 48],
                            stk[:, c * 128:(c + 1) * 128],
                            identb[0:48, 0:48])
                    stk48 = sb.tile([128, NCH * 48], BF16, tag="stk48")
                    nc.vector.tensor_copy(stk48[:], tpb[:])
                    for c in range(NCH):
                        ck = slice(c * K, (c + 1) * K)
                        ds = r80p.tile([128, K], BF16, tag="rhs80")
                        nc.scalar.mul(ds[:], sbf[:, ck], dmt[:, c:c + 1])
                        nc.tensor.matmul(
                            RT_ps[0:48, :], lhsT=stk48[:, c * 48:(c + 1) * 48],
                            rhs=sbf[:, ck], start=(c == 0),
                            stop=(c == NCH - 1), skip_group_check=True)
                        nc.tensor.matmul(
                            RT_ps[64:80, :], lhsT=sbf[:, ck],
                            rhs=sbf[:, ck], start=(c == 0),
                            stop=(c == NCH - 1), skip_group_check=True)
                        nc.tensor.matmul(
                            RT_ps2[:], lhsT=ds[:],
                            rhs=sbf[:, ck], start=(c == 0),
                            stop=(c == NCH - 1), skip_group_check=True)
                    nc.vector.tensor_copy(
                        RallT[0:80, g * K:(g + 1) * K], RT_ps[:])
                    nc.vector.tensor_copy(
                        RallT[96:112, g * K:(g + 1) * K], RT_ps2[:])

                def front_b(g, st):
                    """h1 (into stk), node-major s_pre, softmax, for g."""
                    stkoT, dmt = st["stkoT"], st["dmt"]
                    stk = sb.tile([48, N], BF16, tag="stk")
                    for h in range(2):
                        sl = slice(h * 512, (h + 1) * 512)
                        hp = ps1.tile([HID, 512], F32, tag="ps512",
                                      space="PSUM")
                        nc.tensor.matmul(hp[:], lhsT=tWcomb1[:],
                                         rhs=stkoT[:, sl], start=True,
                                         stop=True)
                        if h == 0:
                            nc.scalar.activation(stk[0:HID, sl], hp[:],
                                                 AF.Identity,
                                                 bias=tbrel1col[:])
                        else:
                            nc.vector.tensor_scalar_add(stk[0:HID, sl], hp[:],
                                                        tbrel1col[:])
                    spre = sb.tile([128, NCH * K], F32, tag="spre")
                    tpa = psT.tile([128, NCH * K], F32, tag="tp", space="PSUM")
                    for c in range(NCH):
                        nc.tensor.matmul(
                            tpa[:, c * K:(c + 1) * K],
                            lhsT=stkoT[:, c * 128:(c + 1) * 128],
                            rhs=tWcombP[:], start=True, stop=True)
                    nc.vector.tensor_tensor(spre[:], tpa[:], tbppbc[:],
                                            op=OP.add)
                    gview = spre[:].rearrange("p (c k) -> p c k", k=K)
                    mx = sb.tile([128, NCH], F32, tag="mx")
                    nc.vector.tensor_reduce(mx[:], gview, axis=AX.X, op=OP.max)
                    s_n = sb.tile([128, NCH * K], F32, tag="s_n")
                    nc.vector.tensor_tensor(
                        out=s_n[:].rearrange("p (c k) -> p c k", k=K),
                        in0=gview,
                        in1=mx[:].rearrange("p (c o) -> p c o", o=1)
                        .to_broadcast([128, NCH, K]),
                        op=OP.subtract)
                    nc.scalar.activation(s_n[:], s_n[:], AF.Exp)
                    sm = sb.tile([128, NCH], F32, tag="sm")
                    nc.vector.tensor_reduce(
                        sm[:], s_n[:].rearrange("p (c k) -> p c k", k=K),
                        axis=AX.X, op=OP.add)
                    rcp = sb.tile([128, NCH], F32, tag="rcp")
                    nc.vector.reciprocal(rcp[:], sm[:])
                    nc.vector.tensor_tensor(rcp[:], rcp[:],
                                            dmt[:, NCH:2 * NCH], op=OP.mult)
                    sbf = sb.tile([128, NCH * K], BF16, tag="sbf")
                    nc.vector.tensor_tensor(
                        out=sbf[:].rearrange("p (c k) -> p c k", k=K),
                        in0=s_n[:].rearrange("p (c k) -> p c k", k=K),
                        in1=rcp[:].rearrange("p (c o) -> p c o", o=1)
                        .to_broadcast([128, NCH, K]),
                        op=OP.mult)
                    st["stk"] = stk
                    st["sbf"] = sbf

                # --- chunk-interleaved software pipeline over graphs ---
                prev = None
                for g in range(G):
                    st = front_a(g)
                    chunk_phase(g, st, prev)
                    if prev is not None:
                        qt_out(prev)
                        rt_phase(g - 1, prev)
                    front_b(g, st)
                    prev = st
                # drain: last graph's QT + RT
                qt_h0 = ps1.tile([K, 512], F32, tag="ps512", space="PSUM")
                qt_h1 = ps1.tile([K, 512], F32, tag="ps512", space="PSUM")
                qt_hp = [qt_h0, qt_h1]
                prev["qt_hp"] = qt_hp
                for c in range(NCH):
                    for h in range(2):
                        sl = slice(h * 512, (h + 1) * 512)
                        nc.tensor.matmul(
                            qt_hp[h][:],
                            lhsT=prev["sbf"][:, c * K:(c + 1) * K],
                            rhs=prev["adjc"][c][:, sl], start=(c == 0),
                            stop=(c == NCH - 1), skip_group_check=True)
                qt_out(prev)
                rt_phase(G - 1, prev)

            # ---------------- batched tail over the G graphs ----------------
            with ExitStack() as ctx:
                sb = ctx.enter_context(tc.tile_pool(name="tail", bufs=1))
                ps = ctx.enter_context(
                    tc.tile_pool(name="tailps", bufs=4, space="PSUM"))

                rl_ps = ps.tile([128, 112], F32, tag="tps", space="PSUM")
                nc.tensor.transpose(rl_ps[:], RallT[:], ident[0:112, 0:112])
                Rall = sb.tile([128, 112], F32, tag="Rall")
                nc.vector.tensor_copy(Rall[:], rl_ps[:])
                r_out = Rall[:, 0:HID]            # out^T [16g+k, ch]
                r_oadj = Rall[:, HID:HID + K]     # out_adj [16g+k, l]
                r_ss = Rall[:, 64:80]
                r_sds = Rall[:, 96:112]

                # --- losses ---
                st3 = sb.tile([128, 3], F32, tag="st3")
                t16 = sb.tile([128, K], F32, tag="t16")
                nc.vector.tensor_tensor(t16[:], r_oadj, teyem[:], op=OP.mult)
                nc.vector.tensor_reduce(st3[:, 0:1], t16[:], axis=AX.X, op=OP.add)
                nc.vector.tensor_tensor(t16[:], r_sds, teyem[:], op=OP.mult)
                nc.vector.tensor_reduce(st3[:, 1:2], t16[:], axis=AX.X, op=OP.add)
                nc.scalar.activation(t16[:], r_ss, AF.Square,
                                     accum_out=st3[:, 2:3])
                red_ps = ps.tile([G, 3], F32, tag="tps", space="PSUM")
                nc.tensor.matmul(red_ps[:], lhsT=tM8[:], rhs=st3[:],
                                 start=True, stop=True)
                red = sb.tile([G, 3], F32, tag="red_sb")
                nc.vector.tensor_copy(red[:], red_ps[:])
                rden = sb.tile([G, 1], F32, tag="rden")
                nc.vector.reciprocal(rden[:], red[:, 1:2])
                lossg = sb.tile([G, 2], F32, tag="lossg")
                nc.vector.tensor_tensor(lossg[:, 0:1], red[:, 0:1], rden[:],
                                        op=OP.mult)
                nc.vector.tensor_scalar_mul(lossg[:, 0:1], lossg[:, 0:1], -1.0)
                ssn = sb.tile([G, 1], F32, tag="ssn")
                nc.scalar.sqrt(ssn[:], red[:, 2:3])
                invssn = sb.tile([G, 1], F32, tag="invssn")
                nc.vector.reciprocal(invssn[:], ssn[:])
                bc_ps = ps.tile([128, 1], F32, tag="tps", space="PSUM")
                nc.tensor.matmul(bc_ps[:], lhsT=tM8T[:], rhs=invssn[:],
                                 start=True, stop=True)
                invbc = sb.tile([128, 1], F32, tag="invbc")
                nc.vector.tensor_copy(invbc[:], bc_ps[:])
                E = sb.tile([128, K], F32, tag="E")
                nc.vector.tensor_scalar_mul(E[:], r_ss, invbc[:])
                nc.vector.tensor_tensor(E[:], E[:], teyem025[:], op=OP.subtract)
                v3 = sb.tile([128, 1], F32, tag="v3")
                nc.scalar.activation(E[:], E[:], AF.Square, accum_out=v3[:])
                o2_ps = ps.tile([G, 1], F32, tag="tps", space="PSUM")
                nc.tensor.matmul(o2_ps[:], lhsT=tM8[:], rhs=v3[:],
                                 start=True, stop=True)
                nc.scalar.sqrt(lossg[:, 1:2], o2_ps[:])
                nc.sync.dma_start(stats_o[:, :], lossg[:])

                # --- h2 chain / classifier ---
                oadj = sb.tile([128, K], F32, tag="oadj")
                nc.vector.tensor_tensor(oadj[:], r_oadj, tinv_eye[:], op=OP.mult)
                rs = sb.tile([128, 1], F32, tag="rs")
                nc.vector.tensor_reduce(rs[:], oadj[:], axis=AX.X, op=OP.add)
                dsq = sb.tile([128, 1], F32, tag="dsq")
                nc.scalar.sqrt(dsq[:], rs[:])
                nc.vector.tensor_scalar_add(dsq[:], dsq[:], EPS)
                invd = sb.tile([128, 1], F32, tag="invd")
                nc.vector.reciprocal(invd[:], dsq[:])
                oadj1 = sb.tile([128, K], F32, tag="oadj1")
                nc.vector.tensor_scalar_mul(oadj1[:], oadj[:], invd[:])
                u_ps = ps.tile([G, K], F32, tag="tps", space="PSUM")
                nc.tensor.matmul(u_ps[:], lhsT=tM8[:], rhs=oadj1[:],
                                 start=True, stop=True)
                u_sb = sb.tile([G, K], F32, tag="u_sb")
                nc.vector.tensor_copy(u_sb[:], u_ps[:])
                bc2_ps = ps.tile([128, K], F32, tag="tps", space="PSUM")
                nc.tensor.matmul(bc2_ps[:], lhsT=tM8T[:], rhs=u_sb[:],
                                 start=True, stop=True)
                ubct = sb.tile([128, K], F32, tag="ubct")
                nc.vector.tensor_tensor(ubct[:], bc2_ps[:], teyem[:], op=OP.mult)
                ubc = sb.tile([128, 1], F32, tag="ubc")
                nc.vector.tensor_reduce(ubc[:], ubct[:], axis=AX.X, op=OP.add)
                U = sb.tile([128, G], F32, tag="U")
                nc.vector.tensor_scalar_mul(U[:], tM8[:], ubc[:])
                outsc = sb.tile([128, HID], F32, tag="outsc")
                nc.vector.tensor_scalar_mul(outsc[:], r_out, invd[:])
                woutT_ps = ps.tile([HID, G], F32, tag="tps", space="PSUM")
                nc.tensor.matmul(woutT_ps[:], lhsT=outsc[:], rhs=U[:],
                                 start=True, stop=True)
                woutT = sb.tile([HID, G], F32, tag="woutT")
                nc.vector.tensor_copy(woutT[:], woutT_ps[:])
                colsT_ps = ps.tile([HID, G], F32, tag="tps", space="PSUM")
                nc.tensor.matmul(colsT_ps[:], lhsT=r_out, rhs=tM8[:],
                                 start=True, stop=True)
                colsT = sb.tile([HID, G], F32, tag="colsT")
                nc.vector.tensor_copy(colsT[:], colsT_ps[:])
                gT_ps = ps.tile([HID, G], F32, tag="tps", space="PSUM")
                nc.tensor.matmul(gT_ps[:], lhsT=tWrel2T[:], rhs=woutT[:],
                                 start=True, stop=False)
                nc.tensor.matmul(gT_ps[:], lhsT=tWroot2T[:], rhs=colsT[:],
                                 start=False, stop=True)
                gTs = sb.tile([HID, G], F32, tag="gTs")
                nc.scalar.activation(gTs[:], gT_ps[:], AF.Identity,
                                     bias=tbrel2s[:])
                preT_ps = ps.tile([HID, G], F32, tag="tps", space="PSUM")
                nc.tensor.matmul(preT_ps[:], lhsT=tW2T[:], rhs=gTs[:],
                                 start=True, stop=True)
                preTs = sb.tile([HID, G], F32, tag="preTs")
                nc.scalar.activation(preTs[:], preT_ps[:], AF.Relu,
                                     bias=tb2col[:])
                logT_ps = ps.tile([OUT, G], F32, tag="tps", space="PSUM")
                nc.tensor.matmul(logT_ps[:], lhsT=tW3T[:], rhs=preTs[:],
                                 start=True, stop=True)
                logTs = sb.tile([OUT, G], F32, tag="logTs")
                nc.scalar.activation(logTs[:], logT_ps[:], AF.Identity,
                                     bias=tb3col[:])
                lgT_ps = ps.tile([G, OUT], F32, tag="tps", space="PSUM")
                nc.tensor.transpose(lgT_ps[:], logTs[:], ident[0:OUT, 0:OUT])
                lgs = sb.tile([G, OUT], F32, tag="lgs")
                nc.vector.tensor_copy(lgs[:], lgT_ps[:])
                lmx = sb.tile([G, 1], F32, tag="lmx")
                nc.vector.tensor_reduce(lmx[:], lgs[:], axis=AX.X, op=OP.max)
                xm = sb.tile([G, OUT], F32, tag="xm")
                nc.vector.tensor_scalar(xm[:], lgs[:], lmx[:], None,
                                        op0=OP.subtract)
                ex = sb.tile([G, OUT], F32, tag="ex")
                esum = sb.tile([G, 1], F32, tag="esum")
                nc.scalar.activation(ex[:], xm[:], AF.Exp, accum_out=esum[:])
                lnz = sb.tile([G, 1], F32, tag="lnz")
                nc.scalar.activation(lnz[:], esum[:], AF.Ln)
                lout = sb.tile([G, OUT], F32, tag="lout")
                nc.vector.tensor_scalar(lout[:], xm[:], lnz[:], None,
                                        op0=OP.subtract)
                nc.sync.dma_start(logits_o[:, :], lout[:])

    nc.compile()
    return nc


# --------------------------------------------------------------------------
# Host-side prep
# --------------------------------------------------------------------------
def _prep(x, edge_index, batch):
    x = np.asarray(x, np.float32)
    batch = np.asarray(batch, np.int64)
    edge_index = np.asarray(edge_index, np.int64)
    total = x.shape[0]

    counts = np.bincount(batch, minlength=B)
    starts = np.concatenate([[0], np.cumsum(counts)[:-1]]).astype(np.int64)
    pos = np.arange(total, dtype=np.int64) - starts[batch]

    if counts.min() == N and counts.max() == N:
        xd = x
        mask = np.ones((B, N), np.float32)
    else:
        ok = pos < N
        tgt = batch * N + pos
        xd = np.zeros((B * N, CIN), np.float32)
        xd[tgt[ok]] = x[ok]
        mask = np.zeros(B * N, np.float32)
        mask[tgt[ok]] = 1.0
        mask = mask.reshape(B, N)

    src, dst = edge_index[0], edge_index[1]
    b_e = batch[src]
    psrc, pdst = pos[src], pos[dst]
    ev = (psrc < N) & (pdst < N)
    if not ev.all():
        b_e, psrc, pdst = b_e[ev], psrc[ev], pdst[ev]

    deg = np.bincount(b_e * N + psrc, minlength=B * N)
    deg = deg.astype(np.float32).reshape(B, N)

    key = ((b_e * N + pdst) * N + psrc).astype(np.int64)
    key.sort()
    first = np.empty(len(key), bool)
    first[0] = True
    np.not_equal(key[1:], key[:-1], out=first[1:])
    uq_pos = np.flatnonzero(first)
    uq = key[uq_pos]
    cnt = np.diff(np.append(uq_pos, len(key))).astype(np.float32)

    row = (uq >> 10).astype(np.int64)     # b*N + pdst
    col = (uq & (N - 1)).astype(np.int16)
    rc = np.bincount(row, minlength=B * N)
    L = int(max(rc.max(), 2))
    L += L % 2
    rstart = np.concatenate([[0], np.cumsum(rc)[:-1]])
    offs = np.arange(len(uq)) - rstart[row]
    ell_idx = np.full((B * N, L), -1, np.int16)
    ell_dat = np.zeros((B * N, L), ml_dtypes.bfloat16)
    ell_idx[row, offs] = col
    ell_dat[row, offs] = cnt.astype(ml_dtypes.bfloat16)

    return xd, mask, deg, ell_idx, ell_dat, L


def _weight_consts(ins):
    f32 = np.float32
    eye = np.tile(np.eye(K, dtype=f32), (128 // K, 1))
    m8 = np.zeros((128, G), f32)
    m8[np.arange(128), np.arange(128) // K] = 1.0
    c = dict(

        Wrel2T=ins["Wrel2"].T,
        Wroot2T=ins["Wroot2"].T,
        W2T=ins["W2"].T,
        W3T=ins["W3"].T,

        b1col=ins["b1"][:, None],
        brel1col=ins["brel1"][:, None],
        bppbc=np.tile(ins["bp"] + ins["Wp"] @ ins["brel1"], (128, NCH)),
        brel2s=(K * ins["brel2"])[:, None],
        b2col=ins["b2"][:, None],
        b3col=ins["b3"][:, None],
        eyem=eye,
        eyem025=eye * 0.25,
        inv_eye=1.0 - eye,
        M8=m8,
        M8T=m8.T,
    )
    out = {k: np.ascontiguousarray(np.asarray(v, f32)) for k, v in c.items()}
    bf = ml_dtypes.bfloat16
    out["b1row"] = np.ascontiguousarray(ins["b1"][None, :].astype(bf))
    out["Wcomb1"] = np.ascontiguousarray(np.concatenate(
        [ins["Wrel1"].T, ins["Wroot1"].T], axis=0).astype(bf))
    out["WcombP"] = np.ascontiguousarray(np.concatenate(
        [(ins["Wp"] @ ins["Wrel1"]).T,
         (ins["Wp"] @ ins["Wroot1"]).T], axis=0).astype(bf))
    return out


def _install_profhook():
    try:
        import antenv
        if not hasattr(antenv, "axon_hooks"):
            mod = types.ModuleType("antenv.axon_hooks")
            mod._hook = None
            mod.set_axon_ntff_profile_hook = lambda h: setattr(mod, "_hook", h)
            mod.get_axon_ntff_profile_hook = lambda: mod._hook
            sys.modules["antenv.axon_hooks"] = mod
            antenv.axon_hooks = mod
        from trn_agent_boot.trn_boot import _ntff_profile_via_ctypes
        hook = _ntff_profile_via_ctypes("/opt/axon/libaxon_pjrt.so")
        if hook is not None:
            sys.modules["antenv.axon_hooks"].set_axon_ntff_profile_hook(hook)
            return True
    except Exception:
        pass
    return False


def _run(inputs, trace=False):
    ins = {k: np.asarray(v) for k, v in inputs.items()}
    xd, mask, deg, ell_idx, ell_dat, L = _prep(
        ins["x"], ins["edge_index"], ins["batch"])
    consts = _weight_consts(ins)

    if L not in _COMPILED:
        _COMPILED[L] = _build(L)
    nc = _COMPILED[L]

    xdT = np.ascontiguousarray(xd.T)                      # [CIN, B*N]
    # ELL rows for graph g, partition p, chunk c at [g, p, c*L:(c+1)*L]
    ell_idx = np.ascontiguousarray(
        ell_idx.reshape(B, NCH, 128, L).transpose(0, 2, 1, 3).reshape(
            B, 128, NCH * L))
    ell_dat = np.ascontiguousarray(
        ell_dat.reshape(B, NCH, 128, L).transpose(0, 2, 1, 3).reshape(
            B, 128, NCH * L))
    dm = np.concatenate(
        [deg.reshape(B, NCH, 128).transpose(0, 2, 1),
         mask.reshape(B, NCH, 128).transpose(0, 2, 1)], axis=2)
    dm = np.ascontiguousarray(dm)                         # [B, 128, 16]

    in_maps = []
    for core in range(NCORES):
        gsl = slice(core * G, (core + 1) * G)
        m = dict(
            xsT=np.ascontiguousarray(
                xdT[:, core * G * N:(core + 1) * G * N].astype(
                    ml_dtypes.bfloat16)),
            W1Tb=np.ascontiguousarray(
                ins["W1"].T.astype(ml_dtypes.bfloat16)),
            ell_idx=ell_idx[gsl],
            ell_dat=ell_dat[gsl],
            deg_row=np.ascontiguousarray(deg[gsl].astype(ml_dtypes.bfloat16)),
            dm_ch=dm[gsl],
            **consts,
        )
        in_maps.append(m)

    if trace:
        _install_profhook()
    res = run_bass_kernel_spmd(nc, in_maps, core_ids=list(range(NCORES)),
                               trace=trace)
    logits = np.concatenate([r["logits"] for r in res.results], axis=0)
    stats = np.concatenate([r["stats"] for r in res.results], axis=0)
    mincut = np.float32(stats[:, 0].mean())
    ortho = np.float32(stats[:, 1].mean())
    return (logits, mincut, ortho), res


def kernel(**inputs):
    out, _ = _run(inputs, trace=False)
    return out


# revision 24
# speedup vs baseline: 1.3700x; 1.1140x over previous
"""MinCutNet (dense_mincut_pool GNN) forward on 8 Trainium2 NeuronCores.

Self-contained: takes the full inputs of reference.setup_inputs(), shards the
64 graphs across 8 cores (8 graphs each), runs a Bass/Tile kernel per core,
and gathers (log_softmax logits [64,10], mincut_loss, ortho_loss).

Device strategy per core:
  - The transposed adjacency adjT[j,i] (j=dst,i=src, duplicate edges counted)
    is materialized chunk-by-chunk ([128, 1024] bf16) directly in SBUF with
    gpsimd.local_scatter from host-prepared per-row ELL lists.
  - All "wide" tensors are kept channel-major ([ch, node]) so adjacency
    chunks stream through the TensorEngine as matmul rhs:
        P^T = (adj@h0)^T = sum_c h0[c]^T @ adjT[c]      (+ b1 (x) deg outer)
        Q^T = (adj@s)^T  = sum_c s[c]^T  @ adjT[c]
    Biases are applied as per-partition activation biases at PSUM->SBUF copy.
  - Pooling results for each graph are collected as one accumulated matmul
    R^T[80,16] = [h1 | Q | s | deg*s]^T @ s, stacked into RallT[80,128]
    (free-dim stacking), transposed once, and the loss/classifier tail is
    computed batched over all 8 graphs ([128=8*16, *] tiles).
"""

import sys
import types
from contextlib import ExitStack

import numpy as np
import ml_dtypes

import concourse.bass as bass
import concourse.bacc as bacc
import concourse.tile as tile
import concourse.mybir as mybir
from concourse.bass_utils import run_bass_kernel_spmd
from concourse.masks import make_identity

F32 = mybir.dt.float32
BF16 = mybir.dt.bfloat16
I16 = mybir.dt.int16
AX = mybir.AxisListType
OP = mybir.AluOpType
AF = mybir.ActivationFunctionType

B, N, CIN, HID, K, OUT = 64, 1024, 128, 32, 16, 10
NCORES = 8
G = B // NCORES          # graphs per core
NCH = N // 128           # 128-row chunks per graph
EPS = 1e-15

_COMPILED = {}           # L -> compiled Bacc program


# --------------------------------------------------------------------------
# Bass program
# --------------------------------------------------------------------------
def _build(L):
    nc = bacc.Bacc("TRN2", target_bir_lowering=False, debug=False,
                   enable_asserts=False, num_devices=NCORES)

    def inp(name, shape, dt):
        return nc.dram_tensor(name, shape, dt, kind="ExternalInput").ap()

    xsT = inp("xsT", [CIN, G * N], BF16)
    ell_idx = inp("ell_idx", [G, 128, NCH * L], I16)
    ell_dat = inp("ell_dat", [G, 128, NCH * L], BF16)
    deg_row = inp("deg_row", [G, N], BF16)
    dm_ch = inp("dm_ch", [G, 128, 2 * NCH], F32)   # cols 0:8 deg, 8:16 mask
    W1Tb = inp("W1Tb", [CIN, HID], BF16)
    Wcomb1 = inp("Wcomb1", [2 * HID, HID], BF16)  # [Wrel1.T ; Wroot1.T]
    WcombP = inp("WcombP", [2 * HID, K], BF16)    # [(WpWrel1).T ; (WpWroot1).T]
    Wrel2T = inp("Wrel2T", [HID, HID], F32)
    Wroot2T = inp("Wroot2T", [HID, HID], F32)
    W2T = inp("W2T", [HID, HID], F32)
    W3T = inp("W3T", [HID, OUT], F32)
    b1row = inp("b1row", [1, HID], BF16)
    b1col = inp("b1col", [HID, 1], F32)
    brel1col = inp("brel1col", [HID, 1], F32)
    bppbc = inp("bppbc", [128, NCH * K], F32)     # tiled bp + Wp@brel1
    brel2s = inp("brel2s", [HID, 1], F32)     # 16 * brel2
    b2col = inp("b2col", [HID, 1], F32)
    b3col = inp("b3col", [OUT, 1], F32)
    eyem = inp("eyem", [128, K], F32)         # tiled 16x16 eye
    eyem025 = inp("eyem025", [128, K], F32)   # eye / 4
    inv_eye = inp("inv_eye", [128, K], F32)   # 1 - eye
    M8 = inp("M8", [128, G], F32)             # M8[p,g] = (p//16 == g)
    M8T = inp("M8T", [G, 128], F32)

    logits_o = nc.dram_tensor("logits", [G, OUT], F32, kind="ExternalOutput").ap()
    stats_o = nc.dram_tensor("stats", [G, 2], F32, kind="ExternalOutput").ap()

    with tile.TileContext(nc) as tc:
        with ExitStack() as octx:
            cpool = octx.enter_context(tc.tile_pool(name="consts", bufs=1))
            opool = octx.enter_context(tc.tile_pool(name="outer", bufs=1))

            def cload(ap_in, shape):
                t = cpool.tile(shape, F32, tag=f"c{ap_in.tensor.name}")
                nc.sync.dma_start(t[:], ap_in)
                return t

            ident = cpool.tile([128, 128], F32, tag="ident")
            make_identity(nc, ident[:])
            identb = cpool.tile([128, 128], BF16, tag="identb")
            nc.vector.tensor_copy(identb[:], ident[:])

            tW1Tb = cpool.tile([CIN, HID], BF16, tag="cW1Tb")
            nc.sync.dma_start(tW1Tb[:], W1Tb)
            tWcomb1 = cpool.tile([2 * HID, HID], BF16, tag="cWcomb1")
            nc.sync.dma_start(tWcomb1[:], Wcomb1)
            tWcombP = cpool.tile([2 * HID, K], BF16, tag="cWcombP")
            nc.sync.dma_start(tWcombP[:], WcombP)
            tWrel2T = cload(Wrel2T, [HID, HID])
            tWroot2T = cload(Wroot2T, [HID, HID])
            tW2T = cload(W2T, [HID, HID])
            tW3T = cload(W3T, [HID, OUT])
            tb1row = cpool.tile([1, HID], BF16, tag="cb1row")
            nc.sync.dma_start(tb1row[:], b1row)
            tb1col = cload(b1col, [HID, 1])
            tbrel1col = cload(brel1col, [HID, 1])
            tbppbc = cload(bppbc, [128, NCH * K])
            tbrel2s = cload(brel2s, [HID, 1])
            tb2col = cload(b2col, [HID, 1])
            tb3col = cload(b3col, [OUT, 1])
            teyem = cload(eyem, [128, K])
            teyem025 = cload(eyem025, [128, K])
            tinv_eye = cload(inv_eye, [128, K])
            tM8 = cload(M8, [128, G])
            tM8T = cload(M8T, [G, 128])

            RallT = opool.tile([112, 128], F32, tag="RallT")

            # ---------------- main per-graph loop ----------------
            with ExitStack() as ctx:
                sb = ctx.enter_context(tc.tile_pool(name="sb", bufs=3))
                adjp = ctx.enter_context(tc.tile_pool(name="adj", bufs=3 * NCH))
                ellp = ctx.enter_context(tc.tile_pool(name="ell", bufs=1))
                r80p = ctx.enter_context(tc.tile_pool(name="r80", bufs=4))
                ps1 = ctx.enter_context(
                    tc.tile_pool(name="ps1", bufs=4, space="PSUM"))
                psT = ctx.enter_context(
                    tc.tile_pool(name="psT", bufs=2, space="PSUM"))
                psR = ctx.enter_context(
                    tc.tile_pool(name="psR", bufs=1, space="PSUM"))

                # prefetch all per-graph inputs up front (fits in SBUF,
                # keeps POOL/PE from ever waiting on input DMAs)
                prep = ctx.enter_context(tc.tile_pool(name="prep", bufs=1))
                pre = []
                for g in range(G):
                    eit = ellp.tile([128, NCH * L], I16, tag=f"eidx{g}")
                    edt = ellp.tile([128, NCH * L], BF16, tag=f"edat{g}")
                    nc.sync.dma_start(eit[:], ell_idx[g])
                    nc.sync.dma_start(edt[:], ell_dat[g])
                    xbf = prep.tile([128, N], BF16, tag=f"xbf{g}")
                    nc.sync.dma_start(xbf[:], xsT[:, g * N:(g + 1) * N])
                    d_row = prep.tile([1, N], BF16, tag=f"d_row{g}")
                    nc.sync.dma_start(d_row[:], deg_row[g, None, :])
                    dmt = prep.tile([128, 2 * NCH], F32, tag=f"dmt{g}")
                    nc.sync.dma_start(dmt[:], dm_ch[g])
                    pre.append(dict(eit=eit, edt=edt, xbf=xbf, d_row=d_row,
                                    dmt=dmt))

                def front_a(g):
                    """h0 (both layouts) for graph g."""
                    xbf = pre[g]["xbf"]
                    d_row = pre[g]["d_row"]
                    dmt = pre[g]["dmt"]
                    eit = pre[g]["eit"]
                    edt = pre[g]["edt"]

                    # stkoT rows 0:32 = P^T (later), rows 32:64 = h0T
                    stkoT = sb.tile([2 * HID, N], BF16, tag="stkoT")
                    for h in range(2):
                        sl = slice(h * 512, (h + 1) * 512)
                        hp = ps1.tile([HID, 512], F32, tag="ps512",
                                      space="PSUM")
                        nc.tensor.matmul(hp[:], lhsT=tW1Tb[:],
                                         rhs=xbf[:, sl], start=True, stop=True)
                        if h == 0:
                            nc.scalar.activation(stkoT[HID:2 * HID, sl], hp[:],
                                                 AF.Identity, bias=tb1col[:])
                        else:
                            nc.vector.tensor_scalar_add(stkoT[HID:2 * HID, sl],
                                                        hp[:], tb1col[:])
                    h0bf = sb.tile([128, NCH * HID], BF16, tag="h0bf")
                    nba = psT.tile([128, NCH * HID], F32, tag="tp",
                                   space="PSUM")
                    for c in range(NCH):
                        nc.tensor.matmul(
                            nba[:, c * HID:(c + 1) * HID],
                            lhsT=xbf[:, c * 128:(c + 1) * 128],
                            rhs=tW1Tb[:], start=True, stop=True)
                    nc.vector.tensor_copy(h0bf[:], nba[:])
                    return dict(d_row=d_row, dmt=dmt, eit=eit, edt=edt,
                                stkoT=stkoT, h0bf=h0bf)

                def chunk_phase(g, st, prev):
                    """scatter chunks + P^T for g, interleaved with prev's
                    Q^T so the PE has filler while POOL produces chunks."""
                    adjc = []
                    pt_h0 = ps1.tile([HID, 512], F32, tag="ps512",
                                     space="PSUM")
                    pt_h1 = ps1.tile([HID, 512], F32, tag="ps512",
                                     space="PSUM")
                    pt_hp = [pt_h0, pt_h1]
                    if prev is not None:
                        qt_h0 = ps1.tile([K, 512], F32, tag="ps512",
                                         space="PSUM")
                        qt_h1 = ps1.tile([K, 512], F32, tag="ps512",
                                         space="PSUM")
                        qt_hp = [qt_h0, qt_h1]
                        prev["qt_hp"] = qt_hp
                    for c in range(NCH):
                        at = adjp.tile([128, N], BF16, tag="adj")
                        nc.gpsimd.local_scatter(
                            out_ap=at[:],
                            data_ap=st["edt"][:, c * L:(c + 1) * L],
                            idxs_ap=st["eit"][:, c * L:(c + 1) * L],
                            channels=128, num_elems=N, num_idxs=L)
                        adjc.append(at)
                        if prev is not None:
                            padj = prev["adjc"][c]
                            for h in range(2):
                                sl = slice(h * 512, (h + 1) * 512)
                                nc.tensor.matmul(
                                    qt_hp[h][:],
                                    lhsT=prev["sbf"][:, c * K:(c + 1) * K],
                                    rhs=padj[:, sl], start=(c == 0),
                                    stop=(c == NCH - 1),
                                    skip_group_check=True)
                        for h in range(2):
                            sl = slice(h * 512, (h + 1) * 512)
                            nc.tensor.matmul(
                                pt_hp[h][:],
                                lhsT=st["h0bf"][:, c * HID:(c + 1) * HID],
                                rhs=at[:, sl], start=(c == 0), stop=False,
                                skip_group_check=True)
                    for h in range(2):
                        sl = slice(h * 512, (h + 1) * 512)
                        nc.tensor.matmul(pt_hp[h][:], lhsT=tb1row[:],
                                         rhs=st["d_row"][:, sl], start=False,
                                         stop=True, skip_group_check=True)
                        if h == 0:
                            nc.scalar.copy(st["stkoT"][0:HID, sl], pt_hp[h][:])
                        else:
                            nc.vector.tensor_copy(st["stkoT"][0:HID, sl],
                                                  pt_hp[h][:])
                    st["adjc"] = adjc

                def qt_out(prev):
                    """copy prev's Q^T psum into stk rows 32:48."""
                    qt_hp = prev["qt_hp"]
                    stk = prev["stk"]
                    nc.scalar.copy(stk[HID:HID + K, 0:512], qt_hp[0][:])
                    nc.vector.tensor_copy(stk[HID:HID + K, 512:1024],
                                          qt_hp[1][:])

                def rt_phase(g, prev):
                    """node-major transposes + pooled R^T for prev graph.
                    R rows: 0:32 h1 | 32:48 Q | 64:80 s | 96:112 deg*s
                    (no mask needed on h1: s is already masked, mask^2=mask)"""
                    stk, sbf, dmt = prev["stk"], prev["sbf"], prev["dmt"]
                    RT_ps = psR.tile([80, K], F32, tag="psr", space="PSUM")
                    RT_ps2 = psR.tile([K, K], F32, tag="psr2", space="PSUM")
                    tpb = psT.tile([128, NCH * 48], BF16, tag="tp",
                                   space="PSUM")
                    for c in range(NCH):
                        nc.tensor.transpose(
                            tpb[:, c * 48:(c + 1) * 48],
                            stk[:, c * 128:(c + 1) * 128],
                            identb[0:48, 0:48])
                    stk48 = sb.tile([128, NCH * 48], BF16, tag="stk48")
                    nc.vector.tensor_copy(stk48[:], tpb[:])
                    for c in range(NCH):
                        ck = slice(c * K, (c + 1) * K)
                        ds = r80p.tile([128, K], BF16, tag="rhs80")
                        nc.scalar.mul(ds[:], sbf[:, ck], dmt[:, c:c + 1])
                        nc.tensor.matmul(
                            RT_ps[0:48, :], lhsT=stk48[:, c * 48:(c + 1) * 48],
                            rhs=sbf[:, ck], start=(c == 0),
                            stop=(c == NCH - 1), skip_group_check=True)
                        nc.tensor.matmul(
                            RT_ps[64:80, :], lhsT=sbf[:, ck],
                            rhs=sbf[:, ck], start=(c == 0),
                            stop=(c == NCH - 1), skip_group_check=True)
                        nc.tensor.matmul(
                            RT_ps2[:], lhsT=ds[:],
                            rhs=sbf[:, ck], start=(c == 0),
                            stop=(c == NCH - 1), skip_group_check=True)
                    nc.vector.tensor_copy(
                        RallT[0:80, g * K:(g + 1) * K], RT_ps[:])
                    nc.vector.tensor_copy(
                        RallT[96:112, g * K:(g + 1) * K], RT_ps2[:])

                def front_b(g, st):
                    """h1 (into stk), node-major s_pre, softmax, for g."""
                    stkoT, dmt = st["stkoT"], st["dmt"]
                    stk = sb.tile([48, N], BF16, tag="stk")
                    for h in range(2):
                        sl = slice(h * 512, (h + 1) * 512)
                        hp = ps1.tile([HID, 512], F32, tag="ps512",
                                      space="PSUM")
                        nc.tensor.matmul(hp[:], lhsT=tWcomb1[:],
                                         rhs=stkoT[:, sl], start=True,
                                         stop=True)
                        if h == 0:
                            nc.scalar.activation(stk[0:HID, sl], hp[:],
                                                 AF.Identity,
                                                 bias=tbrel1col[:])
                        else:
                            nc.vector.tensor_scalar_add(stk[0:HID, sl], hp[:],
                                                        tbrel1col[:])
                    spre = sb.tile([128, NCH * K], F32, tag="spre")
                    tpa = psT.tile([128, NCH * K], F32, tag="tp", space="PSUM")
                    for c in range(NCH):
                        nc.tensor.matmul(
                            tpa[:, c * K:(c + 1) * K],
                            lhsT=stkoT[:, c * 128:(c + 1) * 128],
                            rhs=tWcombP[:], start=True, stop=True)
                    nc.vector.tensor_tensor(spre[:], tpa[:], tbppbc[:],
                                            op=OP.add)
                    s_n = sb.tile([128, NCH * K], F32, tag="s_n")
                    nc.scalar.activation(s_n[:], spre[:], AF.Exp)
                    sm = sb.tile([128, NCH], F32, tag="sm")
                    nc.vector.tensor_reduce(
                        sm[:], s_n[:].rearrange("p (c k) -> p c k", k=K),
                        axis=AX.X, op=OP.add)
                    rcp = sb.tile([128, NCH], F32, tag="rcp")
                    nc.vector.reciprocal(rcp[:], sm[:])
                    nc.vector.tensor_tensor(rcp[:], rcp[:],
                                            dmt[:, NCH:2 * NCH], op=OP.mult)
                    sbf = sb.tile([128, NCH * K], BF16, tag="sbf")
                    nc.vector.tensor_tensor(
                        out=sbf[:].rearrange("p (c k) -> p c k", k=K),
                        in0=s_n[:].rearrange("p (c k) -> p c k", k=K),
                        in1=rcp[:].rearrange("p (c o) -> p c o", o=1)
                        .to_broadcast([128, NCH, K]),
                        op=OP.mult)
                    st["stk"] = stk
                    st["sbf"] = sbf

                # --- chunk-interleaved software pipeline over graphs ---
                # front_a runs two graphs ahead and rt_phase trails front_b so
                # the PE always has ready work during graph g's softmax chain.
                sts = {0: front_a(0)}
                if G > 1:
                    sts[1] = front_a(1)
                for g in range(G):
                    prev = sts.get(g - 1)
                    chunk_phase(g, sts[g], prev)
                    if prev is not None:
                        qt_out(prev)
                    front_b(g, sts[g])
                    if prev is not None:
                        rt_phase(g - 1, prev)
                        del sts[g - 1]
                    if g + 2 < G:
                        sts[g + 2] = front_a(g + 2)
                prev = sts[G - 1]
                # drain: last graph's QT + RT
                qt_h0 = ps1.tile([K, 512], F32, tag="ps512", space="PSUM")
                qt_h1 = ps1.tile([K, 512], F32, tag="ps512", space="PSUM")
                qt_hp = [qt_h0, qt_h1]
                prev["qt_hp"] = qt_hp
                for c in range(NCH):
                    for h in range(2):
                        sl = slice(h * 512, (h + 1) * 512)
                        nc.tensor.matmul(
                            qt_hp[h][:],
                            lhsT=prev["sbf"][:, c * K:(c + 1) * K],
                            rhs=prev["adjc"][c][:, sl], start=(c == 0),
                            stop=(c == NCH - 1), skip_group_check=True)
                qt_out(prev)
                rt_phase(G - 1, prev)

            # ---------------- batched tail over the G graphs ----------------
            with ExitStack() as ctx:
                sb = ctx.enter_context(tc.tile_pool(name="tail", bufs=1))
                ps = ctx.enter_context(
                    tc.tile_pool(name="tailps", bufs=4, space="PSUM"))

                rl_ps = ps.tile([128, 112], F32, tag="tps", space="PSUM")
                nc.tensor.transpose(rl_ps[:], RallT[:], ident[0:112, 0:112])
                Rall = sb.tile([128, 112], F32, tag="Rall")
                nc.vector.tensor_copy(Rall[:], rl_ps[:])
                r_out = Rall[:, 0:HID]            # out^T [16g+k, ch]
                r_oadj = Rall[:, HID:HID + K]     # out_adj [16g+k, l]
                r_ss = Rall[:, 64:80]
                r_sds = Rall[:, 96:112]

                # --- losses ---
                st3 = sb.tile([128, 3], F32, tag="st3")
                t16 = sb.tile([128, K], F32, tag="t16")
                nc.vector.tensor_tensor(t16[:], r_oadj, teyem[:], op=OP.mult)
                nc.vector.tensor_reduce(st3[:, 0:1], t16[:], axis=AX.X, op=OP.add)
                nc.vector.tensor_tensor(t16[:], r_sds, teyem[:], op=OP.mult)
                nc.vector.tensor_reduce(st3[:, 1:2], t16[:], axis=AX.X, op=OP.add)
                nc.scalar.activation(t16[:], r_ss, AF.Square,
                                     accum_out=st3[:, 2:3])
                red_ps = ps.tile([G, 3], F32, tag="tps", space="PSUM")
                nc.tensor.matmul(red_ps[:], lhsT=tM8[:], rhs=st3[:],
                                 start=True, stop=True)
                red = sb.tile([G, 3], F32, tag="red_sb")
                nc.vector.tensor_copy(red[:], red_ps[:])
                rden = sb.tile([G, 1], F32, tag="rden")
                nc.vector.reciprocal(rden[:], red[:, 1:2])
                lossg = sb.tile([G, 2], F32, tag="lossg")
                nc.vector.tensor_tensor(lossg[:, 0:1], red[:, 0:1], rden[:],
                                        op=OP.mult)
                nc.vector.tensor_scalar_mul(lossg[:, 0:1], lossg[:, 0:1], -1.0)
                ssn = sb.tile([G, 1], F32, tag="ssn")
                nc.scalar.sqrt(ssn[:], red[:, 2:3])
                invssn = sb.tile([G, 1], F32, tag="invssn")
                nc.vector.reciprocal(invssn[:], ssn[:])
                bc_ps = ps.tile([128, 1], F32, tag="tps", space="PSUM")
                nc.tensor.matmul(bc_ps[:], lhsT=tM8T[:], rhs=invssn[:],
                                 start=True, stop=True)
                invbc = sb.tile([128, 1], F32, tag="invbc")
                nc.vector.tensor_copy(invbc[:], bc_ps[:])
                E = sb.tile([128, K], F32, tag="E")
                nc.vector.tensor_scalar_mul(E[:], r_ss, invbc[:])
                nc.vector.tensor_tensor(E[:], E[:], teyem025[:], op=OP.subtract)
                v3 = sb.tile([128, 1], F32, tag="v3")
                nc.scalar.activation(E[:], E[:], AF.Square, accum_out=v3[:])
                o2_ps = ps.tile([G, 1], F32, tag="tps", space="PSUM")
                nc.tensor.matmul(o2_ps[:], lhsT=tM8[:], rhs=v3[:],
                                 start=True, stop=True)
                nc.scalar.sqrt(lossg[:, 1:2], o2_ps[:])
                nc.sync.dma_start(stats_o[:, :], lossg[:])

                # --- h2 chain / classifier ---
                oadj = sb.tile([128, K], F32, tag="oadj")
                nc.vector.tensor_tensor(oadj[:], r_oadj, tinv_eye[:], op=OP.mult)
                rs = sb.tile([128, 1], F32, tag="rs")
                nc.vector.tensor_reduce(rs[:], oadj[:], axis=AX.X, op=OP.add)
                dsq = sb.tile([128, 1], F32, tag="dsq")
                nc.scalar.sqrt(dsq[:], rs[:])
                nc.vector.tensor_scalar_add(dsq[:], dsq[:], EPS)
                invd = sb.tile([128, 1], F32, tag="invd")
                nc.vector.reciprocal(invd[:], dsq[:])
                oadj1 = sb.tile([128, K], F32, tag="oadj1")
                nc.vector.tensor_scalar_mul(oadj1[:], oadj[:], invd[:])
                u_ps = ps.tile([G, K], F32, tag="tps", space="PSUM")
                nc.tensor.matmul(u_ps[:], lhsT=tM8[:], rhs=oadj1[:],
                                 start=True, stop=True)
                u_sb = sb.tile([G, K], F32, tag="u_sb")
                nc.vector.tensor_copy(u_sb[:], u_ps[:])
                bc2_ps = ps.tile([128, K], F32, tag="tps", space="PSUM")
                nc.tensor.matmul(bc2_ps[:], lhsT=tM8T[:], rhs=u_sb[:],
                                 start=True, stop=True)
                ubct = sb.tile([128, K], F32, tag="ubct")
                nc.vector.tensor_tensor(ubct[:], bc2_ps[:], teyem[:], op=OP.mult)
                ubc = sb.tile([128, 1], F32, tag="ubc")
                nc.vector.tensor_reduce(ubc[:], ubct[:], axis=AX.X, op=OP.add)
                U = sb.tile([128, G], F32, tag="U")
                nc.vector.tensor_scalar_mul(U[:], tM8[:], ubc[:])
                outsc = sb.tile([128, HID], F32, tag="outsc")
                nc.vector.tensor_scalar_mul(outsc[:], r_out, invd[:])
                woutT_ps = ps.tile([HID, G], F32, tag="tps", space="PSUM")
                nc.tensor.matmul(woutT_ps[:], lhsT=outsc[:], rhs=U[:],
                                 start=True, stop=True)
                woutT = sb.tile([HID, G], F32, tag="woutT")
                nc.vector.tensor_copy(woutT[:], woutT_ps[:])
                colsT_ps = ps.tile([HID, G], F32, tag="tps", space="PSUM")
                nc.tensor.matmul(colsT_ps[:], lhsT=r_out, rhs=tM8[:],
                                 start=True, stop=True)
                colsT = sb.tile([HID, G], F32, tag="colsT")
                nc.vector.tensor_copy(colsT[:], colsT_ps[:])
                gT_ps = ps.tile([HID, G], F32, tag="tps", space="PSUM")
                nc.tensor.matmul(gT_ps[:], lhsT=tWrel2T[:], rhs=woutT[:],
                                 start=True, stop=False)
                nc.tensor.matmul(gT_ps[:], lhsT=tWroot2T[:], rhs=colsT[:],
                                 start=False, stop=True)
                gTs = sb.tile([HID, G], F32, tag="gTs")
                nc.scalar.activation(gTs[:], gT_ps[:], AF.Identity,
                                     bias=tbrel2s[:])
                preT_ps = ps.tile([HID, G], F32, tag="tps", space="PSUM")
                nc.tensor.matmul(preT_ps[:], lhsT=tW2T[:], rhs=gTs[:],
                                 start=True, stop=True)
                preTs = sb.tile([HID, G], F32, tag="preTs")
                nc.scalar.activation(preTs[:], preT_ps[:], AF.Relu,
                                     bias=tb2col[:])
                logT_ps = ps.tile([OUT, G], F32, tag="tps", space="PSUM")
                nc.tensor.matmul(logT_ps[:], lhsT=tW3T[:], rhs=preTs[:],
                                 start=True, stop=True)
                logTs = sb.tile([OUT, G], F32, tag="logTs")
                nc.scalar.activation(logTs[:], logT_ps[:], AF.Identity,
                                     bias=tb3col[:])
                lgT_ps = ps.tile([G, OUT], F32, tag="tps", space="PSUM")
                nc.tensor.transpose(lgT_ps[:], logTs[:], ident[0:OUT, 0:OUT])
                lgs = sb.tile([G, OUT], F32, tag="lgs")
                nc.vector.tensor_copy(lgs[:], lgT_ps[:])
                lmx = sb.tile([G, 1], F32, tag="lmx")
                nc.vector.tensor_reduce(lmx[:], lgs[:], axis=AX.X, op=OP.max)
                xm = sb.tile([G, OUT], F32, tag="xm")
                nc.vector.tensor_scalar(xm[:], lgs[:], lmx[:], None,
                                        op0=OP.subtract)
                ex = sb.tile([G, OUT], F32, tag="ex")
                esum = sb.tile([G, 1], F32, tag="esum")
                nc.scalar.activation(ex[:], xm[:], AF.Exp, accum_out=esum[:])
                lnz = sb.tile([G, 1], F32, tag="lnz")
                nc.scalar.activation(lnz[:], esum[:], AF.Ln)
                lout = sb.tile([G, OUT], F32, tag="lout")
                nc.vector.tensor_scalar(lout[:], xm[:], lnz[:], None,
                                        op0=OP.subtract)
                nc.sync.dma_start(logits_o[:, :], lout[:])

    nc.compile()
    return nc


# --------------------------------------------------------------------------
# Host-side prep
# --------------------------------------------------------------------------
def _prep(x, edge_index, batch):
    x = np.asarray(x, np.float32)
    batch = np.asarray(batch, np.int64)
    edge_index = np.asarray(edge_index, np.int64)
    total = x.shape[0]

    counts = np.bincount(batch, minlength=B)
    starts = np.concatenate([[0], np.cumsum(counts)[:-1]]).astype(np.int64)
    pos = np.arange(total, dtype=np.int64) - starts[batch]

    if counts.min() == N and counts.max() == N:
        xd = x
        mask = np.ones((B, N), np.float32)
    else:
        ok = pos < N
        tgt = batch * N + pos
        xd = np.zeros((B * N, CIN), np.float32)
        xd[tgt[ok]] = x[ok]
        mask = np.zeros(B * N, np.float32)
        mask[tgt[ok]] = 1.0
        mask = mask.reshape(B, N)

    src, dst = edge_index[0], edge_index[1]
    b_e = batch[src]
    psrc, pdst = pos[src], pos[dst]
    ev = (psrc < N) & (pdst < N)
    if not ev.all():
        b_e, psrc, pdst = b_e[ev], psrc[ev], pdst[ev]

    deg = np.bincount(b_e * N + psrc, minlength=B * N)
    deg = deg.astype(np.float32).reshape(B, N)

    key = ((b_e * N + pdst) * N + psrc).astype(np.int64)
    key.sort()
    first = np.empty(len(key), bool)
    first[0] = True
    np.not_equal(key[1:], key[:-1], out=first[1:])
    uq_pos = np.flatnonzero(first)
    uq = key[uq_pos]
    cnt = np.diff(np.append(uq_pos, len(key))).astype(np.float32)

    row = (uq >> 10).astype(np.int64)     # b*N + pdst
    col = (uq & (N - 1)).astype(np.int16)
    rc = np.bincount(row, minlength=B * N)
    L = int(max(rc.max(), 2))
    L += L % 2
    rstart = np.concatenate([[0], np.cumsum(rc)[:-1]])
    offs = np.arange(len(uq)) - rstart[row]
    ell_idx = np.full((B * N, L), -1, np.int16)
    ell_dat = np.zeros((B * N, L), ml_dtypes.bfloat16)
    ell_idx[row, offs] = col
    ell_dat[row, offs] = cnt.astype(ml_dtypes.bfloat16)

    return xd, mask, deg, ell_idx, ell_dat, L


def _weight_consts(ins):
    f32 = np.float32
    eye = np.tile(np.eye(K, dtype=f32), (128 // K, 1))
    m8 = np.zeros((128, G), f32)
    m8[np.arange(128), np.arange(128) // K] = 1.0
    c = dict(

        Wrel2T=ins["Wrel2"].T,
        Wroot2T=ins["Wroot2"].T,
        W2T=ins["W2"].T,
        W3T=ins["W3"].T,

        b1col=ins["b1"][:, None],
        brel1col=ins["brel1"][:, None],
        bppbc=np.tile(ins["bp"] + ins["Wp"] @ ins["brel1"], (128, NCH)),
        brel2s=(K * ins["brel2"])[:, None],
        b2col=ins["b2"][:, None],
        b3col=ins["b3"][:, None],
        eyem=eye,
        eyem025=eye * 0.25,
        inv_eye=1.0 - eye,
        M8=m8,
        M8T=m8.T,
    )
    out = {k: np.ascontiguousarray(np.asarray(v, f32)) for k, v in c.items()}
    bf = ml_dtypes.bfloat16
    out["b1row"] = np.ascontiguousarray(ins["b1"][None, :].astype(bf))
    out["Wcomb1"] = np.ascontiguousarray(np.concatenate(
        [ins["Wrel1"].T, ins["Wroot1"].T], axis=0).astype(bf))
    out["WcombP"] = np.ascontiguousarray(np.concatenate(
        [(ins["Wp"] @ ins["Wrel1"]).T,
         (ins["Wp"] @ ins["Wroot1"]).T], axis=0).astype(bf))
    return out


def _install_profhook():
    try:
        import antenv
        if not hasattr(antenv, "axon_hooks"):
            mod = types.ModuleType("antenv.axon_hooks")
            mod._hook = None
            mod.set_axon_ntff_profile_hook = lambda h: setattr(mod, "_hook", h)
            mod.get_axon_ntff_profile_hook = lambda: mod._hook
            sys.modules["antenv.axon_hooks"] = mod
            antenv.axon_hooks = mod
        from trn_agent_boot.trn_boot import _ntff_profile_via_ctypes
        hook = _ntff_profile_via_ctypes("/opt/axon/libaxon_pjrt.so")
        if hook is not None:
            sys.modules["antenv.axon_hooks"].set_axon_ntff_profile_hook(hook)
            return True
    except Exception:
        pass
    return False


def _run(inputs, trace=False):
    ins = {k: np.asarray(v) for k, v in inputs.items()}
    xd, mask, deg, ell_idx, ell_dat, L = _prep(
        ins["x"], ins["edge_index"], ins["batch"])
    consts = _weight_consts(ins)

    if L not in _COMPILED:
        _COMPILED[L] = _build(L)
    nc = _COMPILED[L]

    xdT = np.ascontiguousarray(xd.T)                      # [CIN, B*N]
    # ELL rows for graph g, partition p, chunk c at [g, p, c*L:(c+1)*L]
    ell_idx = np.ascontiguousarray(
        ell_idx.reshape(B, NCH, 128, L).transpose(0, 2, 1, 3).reshape(
            B, 128, NCH * L))
    ell_dat = np.ascontiguousarray(
        ell_dat.reshape(B, NCH, 128, L).transpose(0, 2, 1, 3).reshape(
            B, 128, NCH * L))
    dm = np.concatenate(
        [deg.reshape(B, NCH, 128).transpose(0, 2, 1),
         mask.reshape(B, NCH, 128).transpose(0, 2, 1)], axis=2)
    dm = np.ascontiguousarray(dm)                         # [B, 128, 16]

    in_maps = []
    for core in range(NCORES):
        gsl = slice(core * G, (core + 1) * G)
        m = dict(
            xsT=np.ascontiguousarray(
                xdT[:, core * G * N:(core + 1) * G * N].astype(
                    ml_dtypes.bfloat16)),
            W1Tb=np.ascontiguousarray(
                ins["W1"].T.astype(ml_dtypes.bfloat16)),
            ell_idx=ell_idx[gsl],
            ell_dat=ell_dat[gsl],
            deg_row=np.ascontiguousarray(deg[gsl].astype(ml_dtypes.bfloat16)),
            dm_ch=dm[gsl],
            **consts,
        )
        in_maps.append(m)

    if trace:
        _install_profhook()
    res = run_bass_kernel_spmd(nc, in_maps, core_ids=list(range(NCORES)),
                               trace=trace)
    logits = np.concatenate([r["logits"] for r in res.results], axis=0)
    stats = np.concatenate([r["stats"] for r in res.results], axis=0)
    mincut = np.float32(stats[:, 0].mean())
    ortho = np.float32(stats[:, 1].mean())
    return (logits, mincut, ortho), res


def kernel(**inputs):
    out, _ = _run(inputs, trace=False)
    return out
